# revision 3
# baseline (speedup 1.0000x reference)
"""Trainium2 Bass kernel for CSA (3x3 convolutional self-attention).

Reference computation (per sample):
  att = softmax over q of (x @ w_qkv.T) / sqrt(hd), per (head, p)    [N, heads, 9, 9]
  U_q = shifted(x) @ w_v[q].T  (q = 3x3 window position)             [N, C] per q
  out[n, p, c] = sum_q att[n, h(c), p, q] * U_q[n + off_q, c]
  y_pre[m, c]  = sum_p out[m - off_p, p, c]    (fold)
  y = y_pre @ w_proj.T

Distribution: 8 cores = 4 samples x 2 row-halves (64 rows each + 2-row halo).

Per-core software pipeline over source rows s (68 = 64 + 2*2 halo), with
engine balance (steady-state ns/row, cost-model):
  PE  (~5.1us): att matmul; 9 U matmuls; x-row transpose (bf16); fused
      fold+q-reduce (81 shift-matrix matmuls/3 rows into PSUM); y transposes
      + projection, lagged one fold group so PE never waits on ACT.
  DVE (~5.3us): softmax reciprocal; attention-broadcast products for heads
      0-2 and head 3 k<4 (bf16 2x mode), for mult-row m = s-2 (2 rows of
      slack so DVE never waits within the row).
  Pool(~5.3us): softmax sum reduce; mask multiply; attention normalize+
      broadcast (attb); product chunk head 3 k in [4,9).
  ACT (~3.1us): x bf16 cast; exp; PSUM evacuations.
The fold for group g fires 6 rows after its first output row so all product
tiles are at least one row old (PE streams without stalls and holds its
ramped 2.4 GHz p-state).
Image-edge correctness is data-driven via per-row masks (single SPMD graph).
"""

import sys

sys.path.insert(0, "/opt/trn_rl_repo")

import numpy as np

import concourse.bass as bass
import concourse.mybir as mybir
import concourse.tile as tile
from concourse.bass_utils import run_bass_kernel_spmd

F32 = mybir.dt.float32
BF16 = mybir.dt.bfloat16
AF = mybir.ActivationFunctionType

K = 3
K2 = 9
HEADS = 4
C = 128
HD = 32
B, H, W = 4, 128, 128
ROWS = H // 2 + 4  # 68 rows per shard (64 + 2 halo each side)
N_CORES = 8
O324 = K2 * K2 * HEADS  # 324
KPOOL = 3  # head-2 k-positions [KPOOL, 9) + all of head 3 computed on Pool

_CACHE = {}
LAST_RESULTS = None  # test harness can inspect exec_time


def build_graph(repeat=1):
    nc = bass.Bass()

    x_d = nc.declare_dram_parameter("x", [ROWS, W, C], F32, isOutput=False)
    wqkvT_d = nc.declare_dram_parameter("wqkvT", [C, O324], F32, isOutput=False)
    wvT_d = nc.declare_dram_parameter("wvT", [C, K2 * C], F32, isOutput=False)
    wprojT_d = nc.declare_dram_parameter("wprojT", [C, C], F32, isOutput=False)
    shifts_d = nc.declare_dram_parameter("shifts", [W, 3 * W], F32, isOutput=False)
    masks_d = nc.declare_dram_parameter("masks", [W, ROWS], F32, isOutput=False)
    out_d = nc.declare_dram_parameter("out", [H // 2, W, C], F32, isOutput=True)

    from contextlib import ExitStack
    with tile.TileContext(nc) as tc, ExitStack() as es:
        cpool = es.enter_context(tc.tile_pool(name="const", bufs=1))
        spool = es.enter_context(tc.tile_pool(name="stage", bufs=1))
        xpool = es.enter_context(tc.tile_pool(name="xin", bufs=4))
        xbpool = es.enter_context(tc.tile_pool(name="xbf", bufs=3))
        epool = es.enter_context(tc.tile_pool(name="esb", bufs=3))
        smpool = es.enter_context(tc.tile_pool(name="small", bufs=8))
        apool = es.enter_context(tc.tile_pool(name="attb", bufs=4))
        vpool = es.enter_context(tc.tile_pool(name="vprime", bufs=6))
        ppool = es.enter_context(tc.tile_pool(name="prod", bufs=7))
        ypool = es.enter_context(tc.tile_pool(name="ysb", bufs=2))
        ytpool = es.enter_context(tc.tile_pool(name="ytsb", bufs=4))
        fpool = es.enter_context(tc.tile_pool(name="fsb", bufs=2))
        transps = es.enter_context(tc.tile_pool(name="tps", bufs=2, space="PSUM"))
        attps = es.enter_context(tc.tile_pool(name="attps", bufs=1, space="PSUM"))
        ups = es.enter_context(tc.tile_pool(name="ups", bufs=3, space="PSUM"))
        ypreps = es.enter_context(tc.tile_pool(name="ypreps", bufs=1, space="PSUM"))
        finps = es.enter_context(tc.tile_pool(name="finps", bufs=1, space="PSUM"))

        # ---- constants: DMA f32, cast to bf16 where needed.  Ordered so the
        # pipeline can start ASAP: shifts (eye for the first transpose) and
        # the first x rows go before the big weight tensors. ----
        def load_const_bf16(dram_ap, shape, name):
            st = spool.tile(shape, F32, tag=f"stage_{name}", name=f"stage_{name}")
            nc.sync.dma_start(out=st[:], in_=dram_ap)
            t = cpool.tile(shape, BF16, tag=name, name=name)
            nc.vector.tensor_copy(t[:], st[:])
            return t

        shifts = load_const_bf16(shifts_d[:], [W, 3 * W], "shifts")
        wqkvT = load_const_bf16(wqkvT_d[:], [C, O324], "wqkvT")
        wvT = load_const_bf16(wvT_d[:], [C, K2 * C], "wvT")
        wprojT = load_const_bf16(wprojT_d[:], [C, C], "wprojT")
        masks = cpool.tile([W, ROWS], F32, tag="masks")
        nc.sync.dma_start(out=masks[:], in_=masks_d[:])

        eye_bf = shifts[:, W : 2 * W]  # shift b=1 is the identity

        # persistent x-transpose tiles (manual rotation; edge columns are
        # zeroed once and never rewritten -> image border padding)
        xtp = [
            cpool.tile([C, W + 2], BF16, tag=f"xtp{i}", name=f"xtp{i}")
            for i in range(4)
        ]
        for i in range(4):
            nc.gpsimd.memset(xtp[i][:, 0:1], 0.0)
            nc.gpsimd.memset(xtp[i][:, W + 1 : W + 2], 0.0)

        scale = float(HD) ** -0.5

        for rep in range(repeat):
            x_tiles = {}
            v_tiles = {}
            prod_tiles = {}
            attb_tiles = {}
            ysb_tiles = {}
            ytsb_tiles = {}
            fin_tiles = {}

            def load_x(s):
                x_sb = xpool.tile([W, C], F32, tag="x", name=f"x{rep}_{s}")
                nc.sync.dma_start(out=x_sb[:], in_=x_d[s])
                x_tiles[s] = x_sb

            def cast_x(s):
                xb = xbpool.tile([W, C], BF16, tag="xb", name=f"xb{rep}_{s}")
                nc.scalar.copy(xb[:], x_tiles.pop(s)[:])
                return xb

            def transpose_x(s, xb):
                xt_ps = transps.tile([C, W], BF16, tag="tr")
                nc.tensor.transpose(xt_ps[:], xb[:], eye_bf)
                nc.scalar.copy(xtp[s % 4][:, 1 : W + 1], xt_ps[:])

            def get_vtile(t):
                if (rep, t) not in v_tiles:
                    v_tiles[(rep, t)] = vpool.tile(
                        [W, HEADS * K2 * HD], BF16, tag="vp", name=f"vp{rep}_{t}"
                    )
                return v_tiles[(rep, t)]

            def u_matmuls(s):
                # q = a*3 + b ; contributes to mult-row t = s - a + 1
                # One PSUM tile per a (3 q's) so each evacuates in one ACT op.
                xs = xtp[s % 4]
                u_ts = [
                    ups.tile([W, 3 * C], F32, tag="u", name=f"u{rep}_{s}_{a_}")
                    for a_ in range(K)
                ]
                for b in (1, 0, 2):
                    for a in range(K):
                        t = s - a + 1
                        if not (1 <= t <= ROWS - 2):
                            continue
                        q = a * K + b
                        nc.tensor.matmul(
                            u_ts[a][:, b * C : (b + 1) * C],
                            xs[:, b : b + W],
                            wvT[:, q * C : (q + 1) * C],
                            start=True,
                            stop=True,
                        )
                for a in range(K):
                    t = s - a + 1
                    if not (1 <= t <= ROWS - 2):
                        continue
                    vt = get_vtile(t)
                    vdst = vt[:].rearrange(
                        "p (h q d) -> p h q d", h=HEADS, q=K2, d=HD
                    )[:, :, 3 * a : 3 * a + 3, :]
                    usrc = u_ts[a][:].rearrange(
                        "p (q h d) -> p h q d", q=K, h=HEADS, d=HD
                    )
                    nc.scalar.copy(vdst, usrc)

            def att_row(s):
                # PE scores -> ACT exp (softmax tail is emitted in att_tail
                # AFTER the row's products so DVE/Pool queue heads never
                # idle-wait on same-row exp)
                xs = xtp[s % 4]
                att_ps = attps.tile([W, O324], F32, tag="att")
                nc.tensor.matmul(
                    att_ps[:], xs[:, 1 : W + 1], wqkvT[:], start=True, stop=True
                )
                e_sb = epool.tile([W, O324], F32, tag="e", name=f"e{rep}_{s}")
                nc.scalar.activation(e_sb[:], att_ps[:], AF.Exp, scale=scale)
                return e_sb

            def att_tail(s, e_sb):
                # DVE: sum over q (free-axis reduce), reciprocal, attb.
                ev = e_sb[:].rearrange("p (g q) -> p g q", q=K2)
                ssum = smpool.tile([W, 36], F32, tag="ssum")
                nc.vector.tensor_reduce(
                    ssum[:], ev, axis=mybir.AxisListType.X, op=mybir.AluOpType.add
                )
                recip = smpool.tile([W, 36], F32, tag="recip")
                nc.vector.reciprocal(recip[:], ssum[:])
                if s in (1, ROWS - 2):
                    # image top/bottom: zero att rows outside the image
                    # (only these rows can be out of range on any core)
                    recipm = smpool.tile([W, 36], F32, tag="recipm")
                    nc.vector.tensor_scalar_mul(
                        recipm[:], recip[:], masks[:, s : s + 1]
                    )
                    recip = recipm
                attb = apool.tile([W, 36 * K2 * 2], BF16, tag="attb",
                                  name=f"attb{rep}_{s}")
                nc.vector.tensor_tensor(
                    attb[:].rearrange("p (g q u) -> p g q u", g=36, q=K2, u=2),
                    e_sb[:]
                    .rearrange("p (g q) -> p g q", q=K2)[:, :, :, None]
                    .broadcast_to([W, 36, K2, 2]),
                    recip[:][:, :, None, None].broadcast_to([W, 36, K2, 2]),
                    op=mybir.AluOpType.mult,
                )
                attb_tiles[(rep, s)] = attb

            def products(m):
                # prod[px, h, k, q, e, u] = att * V'.  DVE (bf16 2x mode,
                # 0.52 ns/elem): heads 0-1 + h2 k<KPOOL; Pool (0.83 ns/elem):
                # h2 k>=KPOOL + all of h3.
                pt = ppool.tile([W, HEADS * K2 * K2 * HD], BF16, tag="prod",
                                name=f"prod{rep}_{m}")
                prod_tiles[(rep, m)] = pt
                vv = v_tiles[(rep, m)][:].rearrange(
                    "p (h q e u) -> p h q e u", h=HEADS, q=K2, e=HD // 2, u=2
                )
                av = attb_tiles[(rep, m)][:].rearrange(
                    "p (h k q u) -> p h k q u", h=HEADS, k=K2, q=K2, u=2
                )
                pv = pt[:].rearrange(
                    "p (h k q e u) -> p h k q e u",
                    h=HEADS, k=K2, q=K2, e=HD // 2, u=2,
                )

                def emit(eng, h, k0, k1):
                    kn = k1 - k0
                    a_b = av[:, h, k0:k1][:, :, :, None, :].broadcast_to(
                        [W, kn, K2, HD // 2, 2]
                    )
                    v_b = vv[:, h][:, None, :, :, :].broadcast_to(
                        [W, kn, K2, HD // 2, 2]
                    )
                    eng.tensor_tensor(
                        pv[:, h, k0:k1], a_b, v_b, op=mybir.AluOpType.mult
                    )

                emit(nc.vector, 0, 0, K2)
                emit(nc.vector, 1, 0, K2)
                emit(nc.vector, 2, 0, K2)
                emit(nc.gpsimd, 3, 0, K2)

            def fold_group(i0, nrows):
                # fold + q-reduce: 27*(nrows+2) shift matmuls into one PSUM
                # region; first matmul is full-width (start=True covers all
                # row blocks)
                ypre_ps = ypreps.tile([W, nrows * C], F32, tag="ypre",
                                      name=f"ypre{rep}_{i0}")
                ts_ = sorted(
                    range(i0 - 1, i0 + nrows + 1),
                    key=lambda t_: -min(i0 + nrows - 1, t_ + 1) + max(i0, t_ - 1),
                )
                mms = []
                for t in ts_:
                    jlo = max(i0, t - 1)
                    jhi = min(i0 + nrows - 1, t + 1)
                    if jlo > jhi or not (1 <= t <= ROWS - 2):
                        continue
                    pv6 = prod_tiles[(rep, t)][:].rearrange(
                        "p (h a b q d) -> p a h b q d",
                        h=HEADS, a=K, b=K, q=K2, d=HD,
                    )
                    a0 = jlo - t + 1
                    alen = jhi - jlo + 1
                    for b1 in range(K):
                        for q in range(K2):
                            mms.append((t, jlo, a0, alen, b1, q, pv6))
                for n_, (t, jlo, a0, alen, b1, q, pv6) in enumerate(mms):
                    rhs = pv6[:, a0 : a0 + alen, :, b1, q, :]
                    nc.tensor.matmul(
                        ypre_ps[:, (jlo - i0) * C : (jlo - i0 + alen) * C],
                        shifts[:, b1 * W : (b1 + 1) * W],
                        rhs,
                        start=(n_ == 0),
                        stop=(n_ == len(mms) - 1),
                    )
                ypre_sb = ypool.tile([W, nrows * C], BF16, tag="ypre_sb",
                                     name=f"ypre_sb{rep}_{i0}")
                nc.scalar.copy(ypre_sb[:], ypre_ps[:])
                ysb_tiles[(rep, i0)] = (ypre_sb, nrows)

            def yt_transposes(i0):
                ypre_sb, nrows = ysb_tiles[(rep, i0)]
                yts = []
                for r_ in range(nrows):
                    yt_ps = transps.tile([C, W], BF16, tag="tr")
                    nc.tensor.transpose(
                        yt_ps[:], ypre_sb[:, r_ * C : (r_ + 1) * C], eye_bf
                    )
                    yt_sb = ytpool.tile([C, W], BF16, tag="yt_sb")
                    nc.scalar.copy(yt_sb[:], yt_ps[:])
                    yts.append(yt_sb)
                ytsb_tiles[(rep, i0)] = yts

            def proj_group(i0):
                yts = ytsb_tiles.pop((rep, i0))
                nrows = len(yts)
                fin_ps = finps.tile([W, nrows * C], F32, tag="fin",
                                    name=f"fin{rep}_{i0}")
                for r_, yt_sb in enumerate(yts):
                    nc.tensor.matmul(
                        fin_ps[:, r_ * C : (r_ + 1) * C],
                        yt_sb[:],
                        wprojT[:],
                        start=True,
                        stop=True,
                    )
                fin_sb = fpool.tile([W, nrows * C], F32, tag="fin_sb",
                                    name=f"fin_sb{rep}_{i0}")
                nc.scalar.copy(fin_sb[:], fin_ps[:])
                dst = out_d[i0 - 2 : i0 - 2 + nrows]
                nc.sync.dma_start(
                    out=dst.rearrange("r w c -> w r c"),
                    in_=fin_sb[:].rearrange("w (r c) -> w r c", r=nrows),
                )

            # ---- prologue: prefetch + first transpose ----
            load_x(0)
            load_x(1)
            transpose_x(0, cast_x(0))

            # ---- steady-state row loop ----
            for s in range(ROWS):
                if s + 2 < ROWS:
                    load_x(s + 2)
                if s + 1 < ROWS:
                    xb_next = cast_x(s + 1)
                # PE: att + U first (inputs one row old)
                if 1 <= s <= ROWS - 2:
                    e_sb = att_row(s)
                u_matmuls(s)
                # PE: lagged y-transposes (before x-transpose: transps PSUM
                # slots rotate yt0,yt1,yt2,xT with prompt ACT evacuations)
                g = s - 6
                fold_due = s >= 8 and (s - 8) % 3 == 0
                if fold_due and g >= 5:
                    yt_transposes(g - 3)
                if s + 1 < ROWS:
                    transpose_x(s + 1, xb_next)
                # DVE/Pool: products for mult-row m = s - 2
                m = s - 2
                if 1 <= m <= ROWS - 2:
                    products(m)
                # PE: this row's fold, then lagged projection
                if fold_due and g <= ROWS - 7:
                    fold_group(g, 3)
                if fold_due and g >= 5:
                    proj_group(g - 3)
                # softmax tail (after products in DVE/Pool queues)
                if 1 <= s <= ROWS - 2:
                    att_tail(s, e_sb)

            # ---- epilogue: last products, folds, projections ----
            products(ROWS - 2)
            yt_transposes(ROWS - 9)  # i0 = 59
            fold_group(ROWS - 6, 3)  # i0 = 62
            proj_group(ROWS - 9)
            yt_transposes(ROWS - 6)
            fold_group(ROWS - 3, 1)  # i0 = 65 (remainder row)
            proj_group(ROWS - 6)
            yt_transposes(ROWS - 3)
            proj_group(ROWS - 3)

    _dedup_ldweights(nc)
    _split_multi_waits(nc)
    return nc


def _dedup_ldweights(nc):
    """Delete InstLdweights whose weights AP is identical to the previous
    weight load on the PE stream (weights persist in the array). Transposes
    load their own stationary, so they invalidate the tracked state. Waits on
    a deleted LDW move to the next kept instruction."""
    import concourse.mybir as mb

    def apkey(arg):
        t = getattr(arg, "bass_ap", None)
        if t is None:
            return str(arg)
        return (t.tensor.name, t.offset, tuple(map(tuple, t.ap)))

    for f in nc.m.functions:
        for bb in f.blocks:
            last_key = None
            pending_waits = []
            out = []
            for inst in bb.instructions:
                eng = str(getattr(inst, "engine", ""))
                tname = type(inst).__name__
                if not eng.endswith("PE"):
                    out.append(inst)
                    continue
                if tname == "InstLdweights":
                    key = tuple(apkey(a) for a in inst.ins)
                    if key == last_key:
                        si = inst.sync_info
                        if si is not None and si.on_wait:
                            pending_waits.extend(si.on_wait)
                        continue
                    last_key = key
                elif tname == "InstMatmult":
                    if getattr(inst, "is_transpose", False):
                        last_key = None
                else:
                    last_key = None
                if pending_waits:
                    si = inst.sync_info
                    if si is None:
                        inst.sync_info = mb.SyncInfo(
                            on_wait=list(pending_waits), on_update=[]
                        )
                    else:
                        si.on_wait = list(pending_waits) + list(si.on_wait)
                    pending_waits = []
                out.append(inst)
            assert not pending_waits
            bb.instructions[:] = out


def _split_multi_waits(nc, limit=1):
    """Walrus codegen accepts at most one sync-wait per instruction on some
    engine structs. Split extras into same-engine NoOps preceding the
    instruction (in-order queues make sequential waits equivalent)."""
    nid = [0]

    def mknop(inst, wait):
        nid[0] += 1
        return mybir.InstNoOp(
            name=f"I-waitnop-{nid[0]}",
            engine=inst.engine,
            ins=[],
            outs=[],
            sync_info=mybir.SyncInfo(on_wait=[wait], on_update=[]),
        )

    for f in nc.m.functions:
        for bb in f.blocks:
            out = []
            for inst in bb.instructions:
                si = inst.sync_info
                if si is not None and si.on_wait and len(si.on_wait) > limit:
                    waits = list(si.on_wait)
                    for w in waits[:-limit]:
                        out.append(mknop(inst, w))
                    si.on_wait = waits[-limit:]
                out.append(inst)
            bb.instructions[:] = out


def prep_inputs(x, w_qkv, w_v, w_proj):
    """Host-side input prep -> per-core input maps."""
    wqkvT = np.ascontiguousarray(w_qkv.T).astype(np.float32)  # [C, 324]
    # wvT[j, q*C + c] = w_v[q, c, j]
    wvT = np.ascontiguousarray(
        np.transpose(w_v, (2, 0, 1)).reshape(C, K2 * C)
    ).astype(np.float32)
    wprojT = np.ascontiguousarray(w_proj.T).astype(np.float32)  # [c, o]
    # S_b[n', j] = delta(n' == j - b + 1) = eye(k = b - 1)
    shifts = np.concatenate(
        [np.eye(W, k=b - 1, dtype=np.float32) for b in range(3)], axis=1
    )

    in_maps = []
    for core in range(N_CORES):
        bb = core // 2
        half = core % 2
        r0 = half * (H // 2)
        # rows r0-2 .. r0+65 with zero pad outside image
        xs = np.zeros((ROWS, W, C), np.float32)
        lo = max(0, r0 - 2)
        hi = min(H, r0 + H // 2 + 2)
        xs[lo - (r0 - 2) : hi - (r0 - 2)] = x[bb, lo:hi]
        # mask: shard row s = image row r0 - 2 + s ; valid iff 0 <= row < H
        mk = np.zeros((ROWS,), np.float32)
        rows = r0 - 2 + np.arange(ROWS)
        mk[(rows >= 0) & (rows < H)] = 1.0
        masks = np.ascontiguousarray(np.broadcast_to(mk[None, :], (W, ROWS)))
        in_maps.append(
            {
                "x": xs,
                "wqkvT": wqkvT,
                "wvT": wvT,
                "wprojT": wprojT,
                "shifts": shifts,
                "masks": masks,
            }
        )
    return in_maps


def kernel(x, w_qkv, w_v, w_proj, _trace=False):
    global LAST_RESULTS
    if "nc" not in _CACHE:
        _CACHE["nc"] = build_graph()
    nc = _CACHE["nc"]
    in_maps = prep_inputs(
        np.asarray(x, np.float32),
        np.asarray(w_qkv, np.float32),
        np.asarray(w_v, np.float32),
        np.asarray(w_proj, np.float32),
    )
    res = run_bass_kernel_spmd(nc, in_maps, list(range(N_CORES)), trace=_trace)
    LAST_RESULTS = res
    y = np.zeros((B, H, W, C), np.float32)
    for core in range(N_CORES):
        bb = core // 2
        half = core % 2
        r0 = half * (H // 2)
        y[bb, r0 : r0 + H // 2] = res.results[core]["out"]
    return y



# revision 7
# speedup vs baseline: 1.3482x; 1.3482x over previous
"""Trainium2 Bass kernel for CSA (3x3 convolutional self-attention).

Reference computation (per sample):
  att = softmax over q of (x @ w_qkv.T) / sqrt(hd), per (head, p)    [N, heads, 9, 9]
  U_q = shifted(x) @ w_v[q].T  (q = 3x3 window position)             [N, C] per q
  out[n, p, c] = sum_q att[n, h(c), p, q] * U_q[n + off_q, c]
  y_pre[m, c]  = sum_p out[m - off_p, p, c]    (fold)
  y = y_pre @ w_proj.T

Distribution: 8 cores = 4 samples x 2 row-halves (64 rows each + 2-row halo).

Per-core software pipeline over source rows s (68 = 64 + 2*2 halo), with
engine balance (steady-state ns/row, cost-model):
  PE  (~5.1us): att matmul; 9 U matmuls; x-row transpose (bf16); fused
      fold+q-reduce (81 shift-matrix matmuls/3 rows into PSUM); y transposes
      + projection, lagged one fold group so PE never waits on ACT.
  DVE (~5.3us): softmax reciprocal; attention-broadcast products for heads
      0-2 and head 3 k<4 (bf16 2x mode), for mult-row m = s-2 (2 rows of
      slack so DVE never waits within the row).
  Pool(~5.3us): softmax sum reduce; mask multiply; attention normalize+
      broadcast (attb); product chunk head 3 k in [4,9).
  ACT (~3.1us): x bf16 cast; exp; PSUM evacuations.
The fold for group g fires 6 rows after its first output row so all product
tiles are at least one row old (PE streams without stalls and holds its
ramped 2.4 GHz p-state).
Image-edge correctness is data-driven via per-row masks (single SPMD graph).
"""

import sys

sys.path.insert(0, "/opt/trn_rl_repo")

import numpy as np

import concourse.bass as bass
import concourse.mybir as mybir
import concourse.tile as tile
from concourse.bass_utils import run_bass_kernel_spmd

F32 = mybir.dt.float32
BF16 = mybir.dt.bfloat16
AF = mybir.ActivationFunctionType

K = 3
K2 = 9
HEADS = 4
C = 128
HD = 32
B, H, W = 4, 128, 128
ROWS = H // 2 + 4  # 68 rows per shard (64 + 2 halo each side)
N_CORES = 8
O324 = K2 * K2 * HEADS  # 324
POOL_UNITS = 9  # (h,k) product units (288 elems each) assigned to Pool,
                # counting from the end (h3 k8 backwards)

_CACHE = {}
LAST_RESULTS = None  # test harness can inspect exec_time


def build_graph(repeat=1):
    nc = bass.Bass()

    x_d = nc.declare_dram_parameter("x", [ROWS, W, C], F32, isOutput=False)
    wqkvT_d = nc.declare_dram_parameter("wqkvT", [C, O324], F32, isOutput=False)
    wvT_d = nc.declare_dram_parameter("wvT", [C, K2 * C], F32, isOutput=False)
    wprojT_d = nc.declare_dram_parameter("wprojT", [C, C], F32, isOutput=False)
    shifts_d = nc.declare_dram_parameter("shifts", [W, 3 * W], F32, isOutput=False)
    masks_d = nc.declare_dram_parameter("masks", [W, ROWS], F32, isOutput=False)
    out_d = nc.declare_dram_parameter("out", [H // 2, W, C], F32, isOutput=True)

    from contextlib import ExitStack
    with tile.TileContext(nc) as tc, ExitStack() as es:
        cpool = es.enter_context(tc.tile_pool(name="const", bufs=1))
        spool = es.enter_context(tc.tile_pool(name="stage", bufs=1))
        xpool = es.enter_context(tc.tile_pool(name="xin", bufs=4))
        xbpool = es.enter_context(tc.tile_pool(name="xbf", bufs=3))
        epool = es.enter_context(tc.tile_pool(name="esb", bufs=3))
        smpool = es.enter_context(tc.tile_pool(name="small", bufs=8))
        apool = es.enter_context(tc.tile_pool(name="attb", bufs=4))
        vpool = es.enter_context(tc.tile_pool(name="vprime", bufs=6))
        ppool = es.enter_context(tc.tile_pool(name="prod", bufs=7))
        ypool = es.enter_context(tc.tile_pool(name="ysb", bufs=2))
        ytpool = es.enter_context(tc.tile_pool(name="ytsb", bufs=4))
        fpool = es.enter_context(tc.tile_pool(name="fsb", bufs=2))
        transps = es.enter_context(tc.tile_pool(name="tps", bufs=2, space="PSUM"))
        attps = es.enter_context(tc.tile_pool(name="attps", bufs=1, space="PSUM"))
        ups = es.enter_context(tc.tile_pool(name="ups", bufs=3, space="PSUM"))
        ypreps = es.enter_context(tc.tile_pool(name="ypreps", bufs=1, space="PSUM"))
        finps = es.enter_context(tc.tile_pool(name="finps", bufs=1, space="PSUM"))

        # ---- constants: DMA f32, cast to bf16 where needed.  Ordered so the
        # pipeline can start ASAP: shifts (eye for the first transpose) and
        # the first x rows go before the big weight tensors. ----
        def load_const_bf16(dram_ap, shape, name):
            st = spool.tile(shape, F32, tag=f"stage_{name}", name=f"stage_{name}")
            nc.sync.dma_start(out=st[:], in_=dram_ap)
            t = cpool.tile(shape, BF16, tag=name, name=name)
            nc.vector.tensor_copy(t[:], st[:])
            return t

        shifts = load_const_bf16(shifts_d[:], [W, 3 * W], "shifts")
        wqkvT = load_const_bf16(wqkvT_d[:], [C, O324], "wqkvT")
        wvT = load_const_bf16(wvT_d[:], [C, K2 * C], "wvT")
        wprojT = load_const_bf16(wprojT_d[:], [C, C], "wprojT")
        masks = cpool.tile([W, ROWS], F32, tag="masks")
        nc.sync.dma_start(out=masks[:], in_=masks_d[:])

        eye_bf = shifts[:, W : 2 * W]  # shift b=1 is the identity

        # persistent x-transpose tiles (manual rotation; edge columns are
        # zeroed once and never rewritten -> image border padding)
        xtp = [
            cpool.tile([C, W + 2], BF16, tag=f"xtp{i}", name=f"xtp{i}")
            for i in range(4)
        ]
        for i in range(4):
            nc.gpsimd.memset(xtp[i][:, 0:1], 0.0)
            nc.gpsimd.memset(xtp[i][:, W + 1 : W + 2], 0.0)

        scale = float(HD) ** -0.5

        for rep in range(repeat):
            x_tiles = {}
            v_tiles = {}
            prod_tiles = {}
            attb_tiles = {}
            ysb_tiles = {}
            ytsb_tiles = {}
            fin_tiles = {}

            def load_x(s):
                x_sb = xpool.tile([W, C], F32, tag="x", name=f"x{rep}_{s}")
                nc.sync.dma_start(out=x_sb[:], in_=x_d[s])
                x_tiles[s] = x_sb

            def cast_x(s):
                xb = xbpool.tile([W, C], BF16, tag="xb", name=f"xb{rep}_{s}")
                nc.scalar.copy(xb[:], x_tiles.pop(s)[:])
                return xb

            def transpose_x(s, xb):
                xt_ps = transps.tile([C, W], BF16, tag="tr")
                nc.tensor.transpose(xt_ps[:], xb[:], eye_bf)
                nc.scalar.copy(xtp[s % 4][:, 1 : W + 1], xt_ps[:])

            def get_vtile(t):
                if (rep, t) not in v_tiles:
                    v_tiles[(rep, t)] = vpool.tile(
                        [W, HEADS * K2 * HD], BF16, tag="vp", name=f"vp{rep}_{t}"
                    )
                return v_tiles[(rep, t)]

            def u_matmuls(s):
                # q = a*3 + b ; contributes to mult-row t = s - a + 1
                # One PSUM tile per a (3 q's) so each evacuates in one ACT op.
                xs = xtp[s % 4]
                u_ts = [
                    ups.tile([W, 3 * C], F32, tag="u", name=f"u{rep}_{s}_{a_}")
                    for a_ in range(K)
                ]
                for b in (1, 0, 2):
                    for a in range(K):
                        t = s - a + 1
                        if not (1 <= t <= ROWS - 2):
                            continue
                        q = a * K + b
                        nc.tensor.matmul(
                            u_ts[a][:, b * C : (b + 1) * C],
                            xs[:, b : b + W],
                            wvT[:, q * C : (q + 1) * C],
                            start=True,
                            stop=True,
                        )
                for a in range(K):
                    t = s - a + 1
                    if not (1 <= t <= ROWS - 2):
                        continue
                    vt = get_vtile(t)
                    vdst = vt[:].rearrange(
                        "p (h q d) -> p h q d", h=HEADS, q=K2, d=HD
                    )[:, :, 3 * a : 3 * a + 3, :]
                    usrc = u_ts[a][:].rearrange(
                        "p (q h d) -> p h q d", q=K, h=HEADS, d=HD
                    )
                    nc.scalar.copy(vdst, usrc)

            def att_row(s):
                # PE scores -> ACT exp, written bf16 and duplicated (u=2) so
                # every downstream DVE op sees packed [1,2] innermost dims and
                # runs in 2x mode. (softmax tail is emitted in att_tail AFTER
                # the row's products so DVE/Pool queue heads never idle-wait
                # on same-row exp)
                xs = xtp[s % 4]
                att_ps = attps.tile([W, O324], F32, tag="att")
                nc.tensor.matmul(
                    att_ps[:], xs[:, 1 : W + 1], wqkvT[:], start=True, stop=True
                )
                e2 = epool.tile([W, O324 * 2], BF16, tag="e", name=f"e{rep}_{s}")
                nc.scalar.activation(
                    e2[:].rearrange("p (o u) -> p o u", u=2),
                    att_ps[:][:, :, None].broadcast_to([W, O324, 2]),
                    AF.Exp,
                    scale=scale,
                )
                return e2

            def att_tail(s, e2):
                # DVE: sum over q (free-axis reduce on the u=0 lane),
                # reciprocal (bf16, duplicated), attb in 2x mode.
                ev = e2[:].rearrange("p (g q u) -> p g q u", q=K2, u=2)
                evu = e2[:].rearrange("p (g q u) -> p u g q", q=K2, u=2)
                ssum = smpool.tile([W, 36], F32, tag="ssum")
                nc.vector.tensor_reduce(
                    ssum[:],
                    evu[:, 0:1],
                    axis=mybir.AxisListType.X,
                    op=mybir.AluOpType.add,
                )
                recip2 = smpool.tile([W, 36 * 2], BF16, tag="recip")
                with nc.allow_low_precision(reason="softmax recip bf16; tol 2e-2"):
                    nc.vector.reciprocal(
                        recip2[:].rearrange("p (g u) -> p g u", u=2),
                        ssum[:][:, :, None].broadcast_to([W, 36, 2]),
                    )
                if s in (1, ROWS - 2):
                    # image top/bottom: zero att rows outside the image
                    # (only these rows can be out of range on any core)
                    recipm2 = smpool.tile([W, 36 * 2], BF16, tag="recipm")
                    nc.vector.tensor_scalar_mul(
                        recipm2[:], recip2[:], masks[:, s : s + 1]
                    )
                    recip2 = recipm2
                attb = apool.tile([W, 36 * K2 * 2], BF16, tag="attb",
                                  name=f"attb{rep}_{s}")
                nc.vector.tensor_tensor(
                    attb[:].rearrange("p (g q u) -> p g q u", g=36, q=K2, u=2),
                    ev,
                    recip2[:]
                    .rearrange("p (g u) -> p g u", u=2)[:, :, None, :]
                    .broadcast_to([W, 36, K2, 2]),
                    op=mybir.AluOpType.mult,
                )
                attb_tiles[(rep, s)] = attb

            def products(m):
                # prod[px, h, k, q, e, u] = att * V'.  DVE (bf16 2x mode,
                # 0.52 ns/elem): heads 0-1 + h2 k<KPOOL; Pool (0.83 ns/elem):
                # h2 k>=KPOOL + all of h3.
                pt = ppool.tile([W, HEADS * K2 * K2 * HD], BF16, tag="prod",
                                name=f"prod{rep}_{m}")
                prod_tiles[(rep, m)] = pt
                vv = v_tiles[(rep, m)][:].rearrange(
                    "p (h q e u) -> p h q e u", h=HEADS, q=K2, e=HD // 2, u=2
                )
                av = attb_tiles[(rep, m)][:].rearrange(
                    "p (h k q u) -> p h k q u", h=HEADS, k=K2, q=K2, u=2
                )
                pv = pt[:].rearrange(
                    "p (h k q e u) -> p h k q e u",
                    h=HEADS, k=K2, q=K2, e=HD // 2, u=2,
                )

                def emit(eng, h, k0, k1):
                    kn = k1 - k0
                    a_b = av[:, h, k0:k1][:, :, :, None, :].broadcast_to(
                        [W, kn, K2, HD // 2, 2]
                    )
                    v_b = vv[:, h][:, None, :, :, :].broadcast_to(
                        [W, kn, K2, HD // 2, 2]
                    )
                    eng.tensor_tensor(
                        pv[:, h, k0:k1], a_b, v_b, op=mybir.AluOpType.mult
                    )

                # Unit = one (h, k) pair (288 elems). Pool takes the last
                # POOL_UNITS units; DVE the rest.
                dve_until = HEADS * K2 - POOL_UNITS
                for h in range(HEADS):
                    lo, hi = h * K2, h * K2 + K2
                    d_hi = min(hi, dve_until)
                    if d_hi > lo:
                        emit(nc.vector, h, 0, d_hi - lo)
                    p_lo = max(lo, dve_until)
                    if hi > p_lo:
                        emit(nc.gpsimd, h, p_lo - lo, K2)

            def fold_group(i0, nrows):
                # fold + q-reduce: 27*(nrows+2) shift matmuls into one PSUM
                # region; first matmul is full-width (start=True covers all
                # row blocks)
                ypre_ps = ypreps.tile([W, nrows * C], F32, tag="ypre",
                                      name=f"ypre{rep}_{i0}")
                ts_ = sorted(
                    range(i0 - 1, i0 + nrows + 1),
                    key=lambda t_: -min(i0 + nrows - 1, t_ + 1) + max(i0, t_ - 1),
                )
                mms = []
                for t in ts_:
                    jlo = max(i0, t - 1)
                    jhi = min(i0 + nrows - 1, t + 1)
                    if jlo > jhi or not (1 <= t <= ROWS - 2):
                        continue
                    pv6 = prod_tiles[(rep, t)][:].rearrange(
                        "p (h a b q d) -> p a h b q d",
                        h=HEADS, a=K, b=K, q=K2, d=HD,
                    )
                    a0 = jlo - t + 1
                    alen = jhi - jlo + 1
                    for b1 in range(K):
                        for q in range(K2):
                            mms.append((t, jlo, a0, alen, b1, q, pv6))
                for n_, (t, jlo, a0, alen, b1, q, pv6) in enumerate(mms):
                    rhs = pv6[:, a0 : a0 + alen, :, b1, q, :]
                    nc.tensor.matmul(
                        ypre_ps[:, (jlo - i0) * C : (jlo - i0 + alen) * C],
                        shifts[:, b1 * W : (b1 + 1) * W],
                        rhs,
                        start=(n_ == 0),
                        stop=(n_ == len(mms) - 1),
                    )
                ypre_sb = ypool.tile([W, nrows * C], BF16, tag="ypre_sb",
                                     name=f"ypre_sb{rep}_{i0}")
                nc.scalar.copy(ypre_sb[:], ypre_ps[:])
                ysb_tiles[(rep, i0)] = (ypre_sb, nrows)

            def yt_transposes(i0):
                ypre_sb, nrows = ysb_tiles[(rep, i0)]
                yts = []
                for r_ in range(nrows):
                    yt_ps = transps.tile([C, W], BF16, tag="tr")
                    nc.tensor.transpose(
                        yt_ps[:], ypre_sb[:, r_ * C : (r_ + 1) * C], eye_bf
                    )
                    yt_sb = ytpool.tile([C, W], BF16, tag="yt_sb")
                    nc.scalar.copy(yt_sb[:], yt_ps[:])
                    yts.append(yt_sb)
                ytsb_tiles[(rep, i0)] = yts

            def proj_group(i0):
                yts = ytsb_tiles.pop((rep, i0))
                nrows = len(yts)
                fin_ps = finps.tile([W, nrows * C], F32, tag="fin",
                                    name=f"fin{rep}_{i0}")
                for r_, yt_sb in enumerate(yts):
                    nc.tensor.matmul(
                        fin_ps[:, r_ * C : (r_ + 1) * C],
                        yt_sb[:],
                        wprojT[:],
                        start=True,
                        stop=True,
                    )
                fin_sb = fpool.tile([W, nrows * C], F32, tag="fin_sb",
                                    name=f"fin_sb{rep}_{i0}")
                nc.scalar.copy(fin_sb[:], fin_ps[:])
                dst = out_d[i0 - 2 : i0 - 2 + nrows]
                nc.sync.dma_start(
                    out=dst.rearrange("r w c -> w r c"),
                    in_=fin_sb[:].rearrange("w (r c) -> w r c", r=nrows),
                )

            # ---- prologue: prefetch + first transpose ----
            load_x(0)
            load_x(1)
            transpose_x(0, cast_x(0))

            # ---- steady-state row loop ----
            for s in range(ROWS):
                if s + 2 < ROWS:
                    load_x(s + 2)
                if s + 1 < ROWS:
                    xb_next = cast_x(s + 1)
                # PE: att + U first (inputs one row old)
                if 1 <= s <= ROWS - 2:
                    e2_row = att_row(s)
                u_matmuls(s)
                # PE: lagged y-transposes (before x-transpose: transps PSUM
                # slots rotate yt0,yt1,yt2,xT with prompt ACT evacuations)
                g = s - 6
                fold_due = s >= 8 and (s - 8) % 3 == 0
                if fold_due and g >= 5:
                    yt_transposes(g - 3)
                if s + 1 < ROWS:
                    transpose_x(s + 1, xb_next)
                # DVE/Pool: products for mult-row m = s - 2
                m = s - 2
                if 1 <= m <= ROWS - 2:
                    products(m)
                # PE: this row's fold, then lagged projection
                if fold_due and g <= ROWS - 7:
                    fold_group(g, 3)
                if fold_due and g >= 5:
                    proj_group(g - 3)
                # softmax tail (after products in DVE/Pool queues)
                if 1 <= s <= ROWS - 2:
                    att_tail(s, e2_row)

            # ---- epilogue: last products, folds, projections ----
            products(ROWS - 2)
            yt_transposes(ROWS - 9)  # i0 = 59
            fold_group(ROWS - 6, 3)  # i0 = 62
            proj_group(ROWS - 9)
            yt_transposes(ROWS - 6)
            fold_group(ROWS - 3, 1)  # i0 = 65 (remainder row)
            proj_group(ROWS - 6)
            yt_transposes(ROWS - 3)
            proj_group(ROWS - 3)

    _dedup_ldweights(nc)
    _split_multi_waits(nc)
    return nc


def _dedup_ldweights(nc):
    """Delete InstLdweights whose weights AP is identical to the previous
    weight load on the PE stream (weights persist in the array). Transposes
    load their own stationary, so they invalidate the tracked state. Waits on
    a deleted LDW move to the next kept instruction."""
    import concourse.mybir as mb

    def apkey(arg):
        t = getattr(arg, "bass_ap", None)
        if t is None:
            return str(arg)
        return (t.tensor.name, t.offset, tuple(map(tuple, t.ap)))

    for f in nc.m.functions:
        for bb in f.blocks:
            last_key = None
            pending_waits = []
            out = []
            for inst in bb.instructions:
                eng = str(getattr(inst, "engine", ""))
                tname = type(inst).__name__
                if not eng.endswith("PE"):
                    out.append(inst)
                    continue
                if tname == "InstLdweights":
                    key = tuple(apkey(a) for a in inst.ins)
                    if key == last_key:
                        si = inst.sync_info
                        if si is not None and si.on_wait:
                            pending_waits.extend(si.on_wait)
                        continue
                    last_key = key
                elif tname == "InstMatmult":
                    if getattr(inst, "is_transpose", False):
                        last_key = None
                else:
                    last_key = None
                if pending_waits:
                    si = inst.sync_info
                    if si is None:
                        inst.sync_info = mb.SyncInfo(
                            on_wait=list(pending_waits), on_update=[]
                        )
                    else:
                        si.on_wait = list(pending_waits) + list(si.on_wait)
                    pending_waits = []
                out.append(inst)
            assert not pending_waits
            bb.instructions[:] = out


def _split_multi_waits(nc, limit=1):
    """Walrus codegen accepts at most one sync-wait per instruction on some
    engine structs. Split extras into same-engine NoOps preceding the
    instruction (in-order queues make sequential waits equivalent)."""
    nid = [0]

    def mknop(inst, wait):
        nid[0] += 1
        return mybir.InstNoOp(
            name=f"I-waitnop-{nid[0]}",
            engine=inst.engine,
            ins=[],
            outs=[],
            sync_info=mybir.SyncInfo(on_wait=[wait], on_update=[]),
        )

    for f in nc.m.functions:
        for bb in f.blocks:
            out = []
            for inst in bb.instructions:
                si = inst.sync_info
                if si is not None and si.on_wait and len(si.on_wait) > limit:
                    waits = list(si.on_wait)
                    for w in waits[:-limit]:
                        out.append(mknop(inst, w))
                    si.on_wait = waits[-limit:]
                out.append(inst)
            bb.instructions[:] = out


def prep_inputs(x, w_qkv, w_v, w_proj):
    """Host-side input prep -> per-core input maps."""
    wqkvT = np.ascontiguousarray(w_qkv.T).astype(np.float32)  # [C, 324]
    # wvT[j, q*C + c] = w_v[q, c, j]
    wvT = np.ascontiguousarray(
        np.transpose(w_v, (2, 0, 1)).reshape(C, K2 * C)
    ).astype(np.float32)
    wprojT = np.ascontiguousarray(w_proj.T).astype(np.float32)  # [c, o]
    # S_b[n', j] = delta(n' == j - b + 1) = eye(k = b - 1)
    shifts = np.concatenate(
        [np.eye(W, k=b - 1, dtype=np.float32) for b in range(3)], axis=1
    )

    in_maps = []
    for core in range(N_CORES):
        bb = core // 2
        half = core % 2
        r0 = half * (H // 2)
        # rows r0-2 .. r0+65 with zero pad outside image
        xs = np.zeros((ROWS, W, C), np.float32)
        lo = max(0, r0 - 2)
        hi = min(H, r0 + H // 2 + 2)
        xs[lo - (r0 - 2) : hi - (r0 - 2)] = x[bb, lo:hi]
        # mask: shard row s = image row r0 - 2 + s ; valid iff 0 <= row < H
        mk = np.zeros((ROWS,), np.float32)
        rows = r0 - 2 + np.arange(ROWS)
        mk[(rows >= 0) & (rows < H)] = 1.0
        masks = np.ascontiguousarray(np.broadcast_to(mk[None, :], (W, ROWS)))
        in_maps.append(
            {
                "x": xs,
                "wqkvT": wqkvT,
                "wvT": wvT,
                "wprojT": wprojT,
                "shifts": shifts,
                "masks": masks,
            }
        )
    return in_maps


def kernel(x, w_qkv, w_v, w_proj, _trace=False):
    global LAST_RESULTS
    if "nc" not in _CACHE:
        _CACHE["nc"] = build_graph()
    nc = _CACHE["nc"]
    in_maps = prep_inputs(
        np.asarray(x, np.float32),
        np.asarray(w_qkv, np.float32),
        np.asarray(w_v, np.float32),
        np.asarray(w_proj, np.float32),
    )
    res = run_bass_kernel_spmd(nc, in_maps, list(range(N_CORES)), trace=_trace)
    LAST_RESULTS = res
    y = np.zeros((B, H, W, C), np.float32)
    for core in range(N_CORES):
        bb = core // 2
        half = core % 2
        r0 = half * (H // 2)
        y[bb, r0 : r0 + H // 2] = res.results[core]["out"]
    return y



# revision 10
# speedup vs baseline: 954.3125x; 707.8365x over previous
"""Trainium2 Bass kernel for CSA (3x3 convolutional self-attention).

Reference computation (per sample):
  att = softmax over q of (x @ w_qkv.T) / sqrt(hd), per (head, p)    [N, heads, 9, 9]
  U_q = shifted(x) @ w_v[q].T  (q = 3x3 window position)             [N, C] per q
  out[n, p, c] = sum_q att[n, h(c), p, q] * U_q[n + off_q, c]
  y_pre[m, c]  = sum_p out[m - off_p, p, c]    (fold)
  y = y_pre @ w_proj.T

Distribution: 8 cores = 4 samples x 2 row-halves (64 rows each + 2-row halo).

Per-core software pipeline over source rows s (68 = 64 + 2*2 halo), with
engine balance (steady-state ns/row, TimelineSim cost model):
  PE  (~5.0us): att matmul; 9 U matmuls; x-row transpose (bf16); fused
      fold+q-reduce (81 shift-matrix matmuls/3 rows into PSUM); y transposes
      + projection, lagged one fold group so PE never waits on ACT.
  DVE (~5.2us): softmax q-sum (free-axis reduce); reciprocal (bf16, written
      duplicated); attb normalize-broadcast (2x mode thanks to the
      duplicated exp); products for heads 0-2 (bf16 2x mode, 0.52 ns/elem),
      for mult-row m = s-2 (2 rows of slack).
  Pool(~5.3us): products for head 3 (gpsimd TT, ~1.98 ns/elem at the 0.42
      Q7 software efficiency).
  ACT (~3.5us): x bf16 cast; exp (written bf16, duplicated u=2 so all DVE
      consumers see packed [1,2] innermost dims); PSUM evacuations.
The fold for group g fires 6 rows after its first output row so all product
tiles are at least one row old (PE streams without stalls and holds its
ramped 2.4 GHz p-state). Reps are software-pipelined into one flat stream:
rep r's PE-only fold/proj tail (virtual rows 68..74) overlaps rep r+1's
DVE/Pool ramp, making the marginal per-rep cost ~ the steady-state floor.
Image-edge correctness is data-driven via per-row masks (single SPMD graph).
"""

import sys

sys.path.insert(0, "/opt/trn_rl_repo")

import numpy as np

import concourse.bass as bass
import concourse.mybir as mybir
import concourse.tile as tile
from concourse.bass_utils import run_bass_kernel_spmd

F32 = mybir.dt.float32
BF16 = mybir.dt.bfloat16
AF = mybir.ActivationFunctionType

K = 3
K2 = 9
HEADS = 4
C = 128
HD = 32
B, H, W = 4, 128, 128
ROWS = H // 2 + 4  # 68 rows per shard (64 + 2 halo each side)
N_CORES = 8
O324 = K2 * K2 * HEADS  # 324
POOL_UNITS = 9  # (h,k) product units (288 elems each) assigned to Pool,
                # counting from the end (h3 k8 backwards)

_CACHE = {}
LAST_RESULTS = None  # test harness can inspect exec_time


def build_graph(repeat=1):
    nc = bass.Bass()

    x_d = nc.declare_dram_parameter("x", [ROWS, W, C], F32, isOutput=False)
    wqkvT_d = nc.declare_dram_parameter("wqkvT", [C, O324], F32, isOutput=False)
    wvT_d = nc.declare_dram_parameter("wvT", [C, K2 * C], F32, isOutput=False)
    wprojT_d = nc.declare_dram_parameter("wprojT", [C, C], F32, isOutput=False)
    shifts_d = nc.declare_dram_parameter("shifts", [W, 3 * W], F32, isOutput=False)
    masks_d = nc.declare_dram_parameter("masks", [W, ROWS], F32, isOutput=False)
    out_d = nc.declare_dram_parameter("out", [H // 2, W, C], F32, isOutput=True)

    from contextlib import ExitStack
    with tile.TileContext(nc) as tc, ExitStack() as es:
        cpool = es.enter_context(tc.tile_pool(name="const", bufs=1))
        spool = es.enter_context(tc.tile_pool(name="stage", bufs=1))
        xpool = es.enter_context(tc.tile_pool(name="xin", bufs=4))
        xbpool = es.enter_context(tc.tile_pool(name="xbf", bufs=3))
        epool = es.enter_context(tc.tile_pool(name="esb", bufs=3))
        smpool = es.enter_context(tc.tile_pool(name="small", bufs=8))
        apool = es.enter_context(tc.tile_pool(name="attb", bufs=4))
        vpool = es.enter_context(tc.tile_pool(name="vprime", bufs=6))
        ppool = es.enter_context(tc.tile_pool(name="prod", bufs=7))
        ypool = es.enter_context(tc.tile_pool(name="ysb", bufs=2))
        ytpool = es.enter_context(tc.tile_pool(name="ytsb", bufs=4))
        fpool = es.enter_context(tc.tile_pool(name="fsb", bufs=2))
        transps = es.enter_context(tc.tile_pool(name="tps", bufs=2, space="PSUM"))
        attps = es.enter_context(tc.tile_pool(name="attps", bufs=1, space="PSUM"))
        ups = es.enter_context(tc.tile_pool(name="ups", bufs=3, space="PSUM"))
        ypreps = es.enter_context(tc.tile_pool(name="ypreps", bufs=1, space="PSUM"))
        finps = es.enter_context(tc.tile_pool(name="finps", bufs=1, space="PSUM"))

        # ---- constants: DMA f32, cast to bf16 where needed.  Ordered so the
        # pipeline can start ASAP: shifts (eye for the first transpose) and
        # the first x rows go before the big weight tensors. ----
        def load_const_bf16(dram_ap, shape, name):
            st = spool.tile(shape, F32, tag=f"stage_{name}", name=f"stage_{name}")
            nc.sync.dma_start(out=st[:], in_=dram_ap)
            t = cpool.tile(shape, BF16, tag=name, name=name)
            nc.vector.tensor_copy(t[:], st[:])
            return t

        shifts = load_const_bf16(shifts_d[:], [W, 3 * W], "shifts")
        wqkvT = load_const_bf16(wqkvT_d[:], [C, O324], "wqkvT")
        wvT = load_const_bf16(wvT_d[:], [C, K2 * C], "wvT")
        wprojT = load_const_bf16(wprojT_d[:], [C, C], "wprojT")
        masks = cpool.tile([W, ROWS], F32, tag="masks")
        nc.sync.dma_start(out=masks[:], in_=masks_d[:])

        eye_bf = shifts[:, W : 2 * W]  # shift b=1 is the identity

        # persistent x-transpose tiles (manual rotation; edge columns are
        # zeroed once and never rewritten -> image border padding)
        xtp = [
            cpool.tile([C, W + 2], BF16, tag=f"xtp{i}", name=f"xtp{i}")
            for i in range(4)
        ]
        for i in range(4):
            nc.gpsimd.memset(xtp[i][:, 0:1], 0.0)
            nc.gpsimd.memset(xtp[i][:, W + 1 : W + 2], 0.0)

        scale = float(HD) ** -0.5

        # ---- flattened software pipeline across reps: rep r's virtual row
        # v = S - r*ROWS runs from -1 (prefetch) to ROWS+6 (tail folds), so
        # rep r's PE-only fold/proj tail overlaps rep r+1's DVE/Pool ramp ----
        x_tiles = {}
        v_tiles = {}
        prod_tiles = {}
        attb_tiles = {}
        ysb_tiles = {}
        ytsb_tiles = {}

        if True:  # keep indentation of the original rep-loop body
            def load_x(rep, s):
                x_sb = xpool.tile([W, C], F32, tag="x", name=f"x{rep}_{s}")
                nc.sync.dma_start(out=x_sb[:], in_=x_d[s])
                x_tiles[(rep, s)] = x_sb

            def cast_x(rep, s):
                xb = xbpool.tile([W, C], BF16, tag="xb", name=f"xb{rep}_{s}")
                nc.scalar.copy(xb[:], x_tiles.pop((rep, s))[:])
                return xb

            def transpose_x(s, xb):
                xt_ps = transps.tile([C, W], BF16, tag="tr")
                nc.tensor.transpose(xt_ps[:], xb[:], eye_bf)
                nc.scalar.copy(xtp[s % 4][:, 1 : W + 1], xt_ps[:])

            def get_vtile(rep, t):
                if (rep, t) not in v_tiles:
                    v_tiles[(rep, t)] = vpool.tile(
                        [W, HEADS * K2 * HD], BF16, tag="vp", name=f"vp{rep}_{t}"
                    )
                return v_tiles[(rep, t)]

            def u_matmuls(rep, s):
                # q = a*3 + b ; contributes to mult-row t = s - a + 1
                # One PSUM tile per a (3 q's) so each evacuates in one ACT op.
                xs = xtp[s % 4]
                u_ts = [
                    ups.tile([W, 3 * C], F32, tag="u", name=f"u{rep}_{s}_{a_}")
                    for a_ in range(K)
                ]
                for b in (1, 0, 2):
                    for a in range(K):
                        t = s - a + 1
                        if not (1 <= t <= ROWS - 2):
                            continue
                        q = a * K + b
                        nc.tensor.matmul(
                            u_ts[a][:, b * C : (b + 1) * C],
                            xs[:, b : b + W],
                            wvT[:, q * C : (q + 1) * C],
                            start=True,
                            stop=True,
                        )
                for a in range(K):
                    t = s - a + 1
                    if not (1 <= t <= ROWS - 2):
                        continue
                    vt = get_vtile(rep, t)
                    vdst = vt[:].rearrange(
                        "p (h q d) -> p h q d", h=HEADS, q=K2, d=HD
                    )[:, :, 3 * a : 3 * a + 3, :]
                    usrc = u_ts[a][:].rearrange(
                        "p (q h d) -> p h q d", q=K, h=HEADS, d=HD
                    )
                    nc.scalar.copy(vdst, usrc)

            def att_row(rep, s):
                # PE scores -> ACT exp, written bf16 and duplicated (u=2) so
                # every downstream DVE op sees packed [1,2] innermost dims and
                # runs in 2x mode. (softmax tail is emitted in att_tail AFTER
                # the row's products so DVE/Pool queue heads never idle-wait
                # on same-row exp)
                xs = xtp[s % 4]
                att_ps = attps.tile([W, O324], F32, tag="att")
                nc.tensor.matmul(
                    att_ps[:], xs[:, 1 : W + 1], wqkvT[:], start=True, stop=True
                )
                e2 = epool.tile([W, O324 * 2], BF16, tag="e", name=f"e{rep}_{s}")
                nc.scalar.activation(
                    e2[:].rearrange("p (o u) -> p o u", u=2),
                    att_ps[:][:, :, None].broadcast_to([W, O324, 2]),
                    AF.Exp,
                    scale=scale,
                )
                return e2

            def att_tail(rep, s, e2):
                # DVE: sum over q (free-axis reduce on the u=0 lane),
                # reciprocal (bf16, duplicated), attb in 2x mode.
                ev = e2[:].rearrange("p (g q u) -> p g q u", q=K2, u=2)
                evu = e2[:].rearrange("p (g q u) -> p u g q", q=K2, u=2)
                ssum = smpool.tile([W, 36], F32, tag="ssum")
                nc.vector.tensor_reduce(
                    ssum[:],
                    evu[:, 0:1],
                    axis=mybir.AxisListType.X,
                    op=mybir.AluOpType.add,
                )
                recip2 = smpool.tile([W, 36 * 2], BF16, tag="recip")
                with nc.allow_low_precision(reason="softmax recip bf16; tol 2e-2"):
                    nc.vector.reciprocal(
                        recip2[:].rearrange("p (g u) -> p g u", u=2),
                        ssum[:][:, :, None].broadcast_to([W, 36, 2]),
                    )
                if s in (1, ROWS - 2):
                    # image top/bottom: zero att rows outside the image
                    # (only these rows can be out of range on any core)
                    recipm2 = smpool.tile([W, 36 * 2], BF16, tag="recipm")
                    nc.vector.tensor_scalar_mul(
                        recipm2[:], recip2[:], masks[:, s : s + 1]
                    )
                    recip2 = recipm2
                attb = apool.tile([W, 36 * K2 * 2], BF16, tag="attb",
                                  name=f"attb{rep}_{s}")
                nc.vector.tensor_tensor(
                    attb[:].rearrange("p (g q u) -> p g q u", g=36, q=K2, u=2),
                    ev,
                    recip2[:]
                    .rearrange("p (g u) -> p g u", u=2)[:, :, None, :]
                    .broadcast_to([W, 36, K2, 2]),
                    op=mybir.AluOpType.mult,
                )
                attb_tiles[(rep, s)] = attb

            def products(rep, m):
                # prod[px, h, k, q, e, u] = att * V'.  DVE (bf16 2x mode,
                # 0.52 ns/elem): heads 0-1 + h2 k<KPOOL; Pool (0.83 ns/elem):
                # h2 k>=KPOOL + all of h3.
                pt = ppool.tile([W, HEADS * K2 * K2 * HD], BF16, tag="prod",
                                name=f"prod{rep}_{m}")
                prod_tiles[(rep, m)] = pt
                vv = v_tiles[(rep, m)][:].rearrange(
                    "p (h q e u) -> p h q e u", h=HEADS, q=K2, e=HD // 2, u=2
                )
                av = attb_tiles[(rep, m)][:].rearrange(
                    "p (h k q u) -> p h k q u", h=HEADS, k=K2, q=K2, u=2
                )
                pv = pt[:].rearrange(
                    "p (h k q e u) -> p h k q e u",
                    h=HEADS, k=K2, q=K2, e=HD // 2, u=2,
                )

                def emit(eng, h, k0, k1):
                    kn = k1 - k0
                    a_b = av[:, h, k0:k1][:, :, :, None, :].broadcast_to(
                        [W, kn, K2, HD // 2, 2]
                    )
                    v_b = vv[:, h][:, None, :, :, :].broadcast_to(
                        [W, kn, K2, HD // 2, 2]
                    )
                    eng.tensor_tensor(
                        pv[:, h, k0:k1], a_b, v_b, op=mybir.AluOpType.mult
                    )

                # Unit = one (h, k) pair (288 elems). Pool takes the last
                # POOL_UNITS units; DVE the rest.
                dve_until = HEADS * K2 - POOL_UNITS
                for h in range(HEADS):
                    lo, hi = h * K2, h * K2 + K2
                    d_hi = min(hi, dve_until)
                    if d_hi > lo:
                        emit(nc.vector, h, 0, d_hi - lo)
                    p_lo = max(lo, dve_until)
                    if hi > p_lo:
                        emit(nc.gpsimd, h, p_lo - lo, K2)

            def fold_group(rep, i0, nrows):
                # fold + q-reduce: 27*(nrows+2) shift matmuls into one PSUM
                # region; first matmul is full-width (start=True covers all
                # row blocks)
                ypre_ps = ypreps.tile([W, nrows * C], F32, tag="ypre",
                                      name=f"ypre{rep}_{i0}")
                ts_ = sorted(
                    range(i0 - 1, i0 + nrows + 1),
                    key=lambda t_: -min(i0 + nrows - 1, t_ + 1) + max(i0, t_ - 1),
                )
                mms = []
                for t in ts_:
                    jlo = max(i0, t - 1)
                    jhi = min(i0 + nrows - 1, t + 1)
                    if jlo > jhi or not (1 <= t <= ROWS - 2):
                        continue
                    pv6 = prod_tiles[(rep, t)][:].rearrange(
                        "p (h a b q d) -> p a h b q d",
                        h=HEADS, a=K, b=K, q=K2, d=HD,
                    )
                    a0 = jlo - t + 1
                    alen = jhi - jlo + 1
                    for b1 in range(K):
                        for q in range(K2):
                            mms.append((t, jlo, a0, alen, b1, q, pv6))
                for n_, (t, jlo, a0, alen, b1, q, pv6) in enumerate(mms):
                    rhs = pv6[:, a0 : a0 + alen, :, b1, q, :]
                    nc.tensor.matmul(
                        ypre_ps[:, (jlo - i0) * C : (jlo - i0 + alen) * C],
                        shifts[:, b1 * W : (b1 + 1) * W],
                        rhs,
                        start=(n_ == 0),
                        stop=(n_ == len(mms) - 1),
                    )
                ypre_sb = ypool.tile([W, nrows * C], BF16, tag="ypre_sb",
                                     name=f"ypre_sb{rep}_{i0}")
                nc.scalar.copy(ypre_sb[:], ypre_ps[:])
                ysb_tiles[(rep, i0)] = (ypre_sb, nrows)

            def yt_transposes(rep, i0):
                ypre_sb, nrows = ysb_tiles[(rep, i0)]
                yts = []
                for r_ in range(nrows):
                    yt_ps = transps.tile([C, W], BF16, tag="tr")
                    nc.tensor.transpose(
                        yt_ps[:], ypre_sb[:, r_ * C : (r_ + 1) * C], eye_bf
                    )
                    yt_sb = ytpool.tile([C, W], BF16, tag="yt_sb")
                    nc.scalar.copy(yt_sb[:], yt_ps[:])
                    yts.append(yt_sb)
                ytsb_tiles[(rep, i0)] = yts

            def proj_group(rep, i0):
                yts = ytsb_tiles.pop((rep, i0))
                nrows = len(yts)
                fin_ps = finps.tile([W, nrows * C], F32, tag="fin",
                                    name=f"fin{rep}_{i0}")
                for r_, yt_sb in enumerate(yts):
                    nc.tensor.matmul(
                        fin_ps[:, r_ * C : (r_ + 1) * C],
                        yt_sb[:],
                        wprojT[:],
                        start=True,
                        stop=True,
                    )
                fin_sb = fpool.tile([W, nrows * C], F32, tag="fin_sb",
                                    name=f"fin_sb{rep}_{i0}")
                nc.scalar.copy(fin_sb[:], fin_ps[:])
                dst = out_d[i0 - 2 : i0 - 2 + nrows]
                nc.sync.dma_start(
                    out=dst.rearrange("r w c -> w r c"),
                    in_=fin_sb[:].rearrange("w (r c) -> w r c", r=nrows),
                )

            def emit_step(rep, v):
                # one virtual pipeline step of rep: v in [-1, ROWS+6]
                if v == -1:
                    # prologue: prefetch + first transpose
                    load_x(rep, 0)
                    load_x(rep, 1)
                    transpose_x(0, cast_x(rep, 0))
                    return
                if v + 2 < ROWS:
                    load_x(rep, v + 2)
                xb_next = cast_x(rep, v + 1) if v + 1 < ROWS else None
                # PE: att + U first (inputs one row old)
                e2_row = None
                if 1 <= v <= ROWS - 2:
                    e2_row = att_row(rep, v)
                if v < ROWS:
                    u_matmuls(rep, v)
                # PE: lagged y-transposes (before x-transpose: transps PSUM
                # slots rotate yt0,yt1,yt2,xT with prompt ACT evacuations)
                g = v - 6
                fold_due = v >= 8 and (v - 8) % 3 == 0
                if fold_due and 5 <= g and g - 3 <= ROWS - 3:
                    yt_transposes(rep, g - 3)
                if xb_next is not None:
                    transpose_x(v + 1, xb_next)
                # DVE/Pool: products for mult-row m = v - 2
                m = v - 2
                if 1 <= m <= ROWS - 2:
                    products(rep, m)
                # PE: this row's fold, then lagged projection
                if fold_due and g <= ROWS - 3:
                    fold_group(rep, g, 3 if g <= ROWS - 6 else 1)
                if fold_due and 5 <= g and g - 3 <= ROWS - 3:
                    proj_group(rep, g - 3)
                # softmax tail (after products in DVE/Pool queues)
                if e2_row is not None:
                    att_tail(rep, v, e2_row)

            for S in range(-1, (repeat - 1) * ROWS + ROWS + 7):
                for rep in range(repeat):
                    v = S - rep * ROWS
                    if -1 <= v <= ROWS + 6:
                        emit_step(rep, v)

    _dedup_ldweights(nc)
    _split_multi_waits(nc)
    return nc


def _dedup_ldweights(nc):
    """Delete InstLdweights whose weights AP is identical to the previous
    weight load on the PE stream (weights persist in the array). Transposes
    load their own stationary, so they invalidate the tracked state. Waits on
    a deleted LDW move to the next kept instruction."""
    import concourse.mybir as mb

    def apkey(arg):
        t = getattr(arg, "bass_ap", None)
        if t is None:
            return str(arg)
        return (t.tensor.name, t.offset, tuple(map(tuple, t.ap)))

    for f in nc.m.functions:
        for bb in f.blocks:
            last_key = None
            pending_waits = []
            out = []
            for inst in bb.instructions:
                eng = str(getattr(inst, "engine", ""))
                tname = type(inst).__name__
                if not eng.endswith("PE"):
                    out.append(inst)
                    continue
                if tname == "InstLdweights":
                    key = tuple(apkey(a) for a in inst.ins)
                    if key == last_key:
                        si = inst.sync_info
                        if si is not None and si.on_wait:
                            pending_waits.extend(si.on_wait)
                        continue
                    last_key = key
                elif tname == "InstMatmult":
                    if getattr(inst, "is_transpose", False):
                        last_key = None
                else:
                    last_key = None
                if pending_waits:
                    si = inst.sync_info
                    if si is None:
                        inst.sync_info = mb.SyncInfo(
                            on_wait=list(pending_waits), on_update=[]
                        )
                    else:
                        si.on_wait = list(pending_waits) + list(si.on_wait)
                    pending_waits = []
                out.append(inst)
            assert not pending_waits
            bb.instructions[:] = out


def _split_multi_waits(nc, limit=1):
    """Walrus codegen accepts at most one sync-wait per instruction on some
    engine structs. Split extras into same-engine NoOps preceding the
    instruction (in-order queues make sequential waits equivalent)."""
    nid = [0]

    def mknop(inst, wait):
        nid[0] += 1
        return mybir.InstNoOp(
            name=f"I-waitnop-{nid[0]}",
            engine=inst.engine,
            ins=[],
            outs=[],
            sync_info=mybir.SyncInfo(on_wait=[wait], on_update=[]),
        )

    for f in nc.m.functions:
        for bb in f.blocks:
            out = []
            for inst in bb.instructions:
                si = inst.sync_info
                if si is not None and si.on_wait and len(si.on_wait) > limit:
                    waits = list(si.on_wait)
                    for w in waits[:-limit]:
                        out.append(mknop(inst, w))
                    si.on_wait = waits[-limit:]
                out.append(inst)
            bb.instructions[:] = out


def prep_inputs(x, w_qkv, w_v, w_proj):
    """Host-side input prep -> per-core input maps."""
    wqkvT = np.ascontiguousarray(w_qkv.T).astype(np.float32)  # [C, 324]
    # wvT[j, q*C + c] = w_v[q, c, j]
    wvT = np.ascontiguousarray(
        np.transpose(w_v, (2, 0, 1)).reshape(C, K2 * C)
    ).astype(np.float32)
    wprojT = np.ascontiguousarray(w_proj.T).astype(np.float32)  # [c, o]
    # S_b[n', j] = delta(n' == j - b + 1) = eye(k = b - 1)
    shifts = np.concatenate(
        [np.eye(W, k=b - 1, dtype=np.float32) for b in range(3)], axis=1
    )

    in_maps = []
    for core in range(N_CORES):
        bb = core // 2
        half = core % 2
        r0 = half * (H // 2)
        # rows r0-2 .. r0+65 with zero pad outside image
        xs = np.zeros((ROWS, W, C), np.float32)
        lo = max(0, r0 - 2)
        hi = min(H, r0 + H // 2 + 2)
        xs[lo - (r0 - 2) : hi - (r0 - 2)] = x[bb, lo:hi]
        # mask: shard row s = image row r0 - 2 + s ; valid iff 0 <= row < H
        mk = np.zeros((ROWS,), np.float32)
        rows = r0 - 2 + np.arange(ROWS)
        mk[(rows >= 0) & (rows < H)] = 1.0
        masks = np.ascontiguousarray(np.broadcast_to(mk[None, :], (W, ROWS)))
        in_maps.append(
            {
                "x": xs,
                "wqkvT": wqkvT,
                "wvT": wvT,
                "wprojT": wprojT,
                "shifts": shifts,
                "masks": masks,
            }
        )
    return in_maps


def kernel(x, w_qkv, w_v, w_proj, _trace=False):
    global LAST_RESULTS
    if "nc" not in _CACHE:
        _CACHE["nc"] = build_graph()
    nc = _CACHE["nc"]
    in_maps = prep_inputs(
        np.asarray(x, np.float32),
        np.asarray(w_qkv, np.float32),
        np.asarray(w_v, np.float32),
        np.asarray(w_proj, np.float32),
    )
    res = run_bass_kernel_spmd(nc, in_maps, list(range(N_CORES)), trace=_trace)
    LAST_RESULTS = res
    y = np.zeros((B, H, W, C), np.float32)
    for core in range(N_CORES):
        bb = core // 2
        half = core % 2
        r0 = half * (H // 2)
        y[bb, r0 : r0 + H // 2] = res.results[core]["out"]
    return y



# revision 11
# speedup vs baseline: 955.9641x; 1.0017x over previous
"""Trainium2 Bass kernel for CSA (3x3 convolutional self-attention).

Reference computation (per sample):
  att = softmax over q of (x @ w_qkv.T) / sqrt(hd), per (head, p)    [N, heads, 9, 9]
  U_q = shifted(x) @ w_v[q].T  (q = 3x3 window position)             [N, C] per q
  out[n, p, c] = sum_q att[n, h(c), p, q] * U_q[n + off_q, c]
  y_pre[m, c]  = sum_p out[m - off_p, p, c]    (fold)
  y = y_pre @ w_proj.T

Distribution: 8 cores = 4 samples x 2 row-halves (64 rows each + 2-row halo).

Per-core software pipeline over source rows s (68 = 64 + 2*2 halo), with
engine balance (steady-state ns/row, TimelineSim cost model):
  PE  (~5.0us): att matmul; 9 U matmuls; x-row transpose (bf16); fused
      fold+q-reduce (81 shift-matrix matmuls/3 rows into PSUM); y transposes
      + projection, lagged one fold group so PE never waits on ACT.
  DVE (~5.2us): softmax q-sum (free-axis reduce); reciprocal (bf16, written
      duplicated); attb normalize-broadcast (2x mode thanks to the
      duplicated exp); products for heads 0-2 (bf16 2x mode, 0.52 ns/elem),
      for mult-row m = s-2 (2 rows of slack).
  Pool(~5.3us): products for head 3 (gpsimd TT, ~1.98 ns/elem at the 0.42
      Q7 software efficiency).
  ACT (~3.5us): x bf16 cast; exp (written bf16, duplicated u=2 so all DVE
      consumers see packed [1,2] innermost dims); PSUM evacuations.
The fold for group g fires 6 rows after its first output row so all product
tiles are at least one row old (PE streams without stalls and holds its
ramped 2.4 GHz p-state). Reps are software-pipelined into one flat stream:
rep r's PE-only fold/proj tail (virtual rows 68..74) overlaps rep r+1's
DVE/Pool ramp, making the marginal per-rep cost ~ the steady-state floor.
Image-edge correctness is data-driven via per-row masks (single SPMD graph).
"""

import sys

sys.path.insert(0, "/opt/trn_rl_repo")

import numpy as np

import concourse.bass as bass
import concourse.mybir as mybir
import concourse.tile as tile
from concourse.bass_utils import run_bass_kernel_spmd

F32 = mybir.dt.float32
BF16 = mybir.dt.bfloat16
AF = mybir.ActivationFunctionType

K = 3
K2 = 9
HEADS = 4
C = 128
HD = 32
B, H, W = 4, 128, 128
ROWS = H // 2 + 4  # 68 rows per shard (64 + 2 halo each side)
N_CORES = 8
O324 = K2 * K2 * HEADS  # 324
POOL_UNITS = 9  # (h,k) product units (288 elems each) assigned to Pool,
                # counting from the end (h3 k8 backwards)

_CACHE = {}
LAST_RESULTS = None  # test harness can inspect exec_time


def build_graph(repeat=1):
    nc = bass.Bass()

    x_d = nc.declare_dram_parameter("x", [ROWS, W, C], F32, isOutput=False)
    wqkvT_d = nc.declare_dram_parameter("wqkvT", [C, O324], F32, isOutput=False)
    wvT_d = nc.declare_dram_parameter("wvT", [C, K2 * C], F32, isOutput=False)
    wprojT_d = nc.declare_dram_parameter("wprojT", [C, C], F32, isOutput=False)
    shifts_d = nc.declare_dram_parameter("shifts", [W, 3 * W], F32, isOutput=False)
    masks_d = nc.declare_dram_parameter("masks", [W, ROWS], F32, isOutput=False)
    out_d = nc.declare_dram_parameter("out", [H // 2, W, C], F32, isOutput=True)

    from contextlib import ExitStack
    with tile.TileContext(nc) as tc, ExitStack() as es:
        cpool = es.enter_context(tc.tile_pool(name="const", bufs=1))
        spool = es.enter_context(tc.tile_pool(name="stage", bufs=1))
        xpool = es.enter_context(tc.tile_pool(name="xin", bufs=4))
        xbpool = es.enter_context(tc.tile_pool(name="xbf", bufs=3))
        epool = es.enter_context(tc.tile_pool(name="esb", bufs=3))
        smpool = es.enter_context(tc.tile_pool(name="small", bufs=8))
        apool = es.enter_context(tc.tile_pool(name="attb", bufs=4))
        vpool = es.enter_context(tc.tile_pool(name="vprime", bufs=6))
        ppool = es.enter_context(tc.tile_pool(name="prod", bufs=7))
        ypool = es.enter_context(tc.tile_pool(name="ysb", bufs=2))
        ytpool = es.enter_context(tc.tile_pool(name="ytsb", bufs=4))
        fpool = es.enter_context(tc.tile_pool(name="fsb", bufs=2))
        transps = es.enter_context(tc.tile_pool(name="tps", bufs=2, space="PSUM"))
        attps = es.enter_context(tc.tile_pool(name="attps", bufs=1, space="PSUM"))
        ups = es.enter_context(tc.tile_pool(name="ups", bufs=3, space="PSUM"))
        ypreps = es.enter_context(tc.tile_pool(name="ypreps", bufs=1, space="PSUM"))
        finps = es.enter_context(tc.tile_pool(name="finps", bufs=1, space="PSUM"))

        # ---- constants: DMA f32, cast to bf16 where needed.  Ordered so the
        # pipeline can start ASAP: shifts (eye for the first transpose) and
        # the first x rows go before the big weight tensors. ----
        def load_const_bf16(dram_ap, shape, name):
            st = spool.tile(shape, F32, tag=f"stage_{name}", name=f"stage_{name}")
            nc.sync.dma_start(out=st[:], in_=dram_ap)
            t = cpool.tile(shape, BF16, tag=name, name=name)
            nc.vector.tensor_copy(t[:], st[:])
            return t

        shifts = load_const_bf16(shifts_d[:], [W, 3 * W], "shifts")
        wqkvT = load_const_bf16(wqkvT_d[:], [C, O324], "wqkvT")
        wvT = load_const_bf16(wvT_d[:], [C, K2 * C], "wvT")
        wprojT = load_const_bf16(wprojT_d[:], [C, C], "wprojT")
        masks = cpool.tile([W, ROWS], F32, tag="masks")
        nc.sync.dma_start(out=masks[:], in_=masks_d[:])

        eye_bf = shifts[:, W : 2 * W]  # shift b=1 is the identity

        # persistent x-transpose tiles (manual rotation; edge columns are
        # zeroed once and never rewritten -> image border padding)
        xtp = [
            cpool.tile([C, W + 2], BF16, tag=f"xtp{i}", name=f"xtp{i}")
            for i in range(4)
        ]
        for i in range(4):
            nc.gpsimd.memset(xtp[i][:, 0:1], 0.0)
            nc.gpsimd.memset(xtp[i][:, W + 1 : W + 2], 0.0)

        scale = float(HD) ** -0.5

        # ---- flattened software pipeline across reps: rep r's virtual row
        # v = S - r*ROWS runs from -1 (prefetch) to ROWS+6 (tail folds), so
        # rep r's PE-only fold/proj tail overlaps rep r+1's DVE/Pool ramp ----
        x_tiles = {}
        v_tiles = {}
        prod_tiles = {}
        attb_tiles = {}
        ysb_tiles = {}
        ytsb_tiles = {}

        if True:  # keep indentation of the original rep-loop body
            def load_x(rep, s):
                x_sb = xpool.tile([W, C], F32, tag="x", name=f"x{rep}_{s}")
                nc.sync.dma_start(out=x_sb[:], in_=x_d[s])
                x_tiles[(rep, s)] = x_sb

            def cast_x(rep, s):
                xb = xbpool.tile([W, C], BF16, tag="xb", name=f"xb{rep}_{s}")
                nc.scalar.copy(xb[:], x_tiles.pop((rep, s))[:])
                return xb

            def transpose_x(s, xb):
                xt_ps = transps.tile([C, W], BF16, tag="tr")
                nc.tensor.transpose(xt_ps[:], xb[:], eye_bf)
                nc.scalar.copy(xtp[s % 4][:, 1 : W + 1], xt_ps[:])

            def get_vtile(rep, t):
                if (rep, t) not in v_tiles:
                    v_tiles[(rep, t)] = vpool.tile(
                        [W, HEADS * K2 * HD], BF16, tag="vp", name=f"vp{rep}_{t}"
                    )
                return v_tiles[(rep, t)]

            def u_matmuls(rep, s):
                # q = a*3 + b ; contributes to mult-row t = s - a + 1
                # One PSUM tile per a (3 q's) so each evacuates in one ACT op.
                xs = xtp[s % 4]
                u_ts = [
                    ups.tile([W, 3 * C], F32, tag="u", name=f"u{rep}_{s}_{a_}")
                    for a_ in range(K)
                ]
                for b in (1, 0, 2):
                    for a in range(K):
                        t = s - a + 1
                        if not (1 <= t <= ROWS - 2):
                            continue
                        q = a * K + b
                        nc.tensor.matmul(
                            u_ts[a][:, b * C : (b + 1) * C],
                            xs[:, b : b + W],
                            wvT[:, q * C : (q + 1) * C],
                            start=True,
                            stop=True,
                        )
                for a in range(K):
                    t = s - a + 1
                    if not (1 <= t <= ROWS - 2):
                        continue
                    vt = get_vtile(rep, t)
                    vdst = vt[:].rearrange(
                        "p (h q d) -> p h q d", h=HEADS, q=K2, d=HD
                    )[:, :, 3 * a : 3 * a + 3, :]
                    usrc = u_ts[a][:].rearrange(
                        "p (q h d) -> p h q d", q=K, h=HEADS, d=HD
                    )
                    nc.scalar.copy(vdst, usrc)

            def att_row(rep, s):
                # PE scores -> ACT exp, written bf16 and duplicated (u=2) so
                # every downstream DVE op sees packed [1,2] innermost dims and
                # runs in 2x mode. (softmax tail is emitted in att_tail AFTER
                # the row's products so DVE/Pool queue heads never idle-wait
                # on same-row exp)
                xs = xtp[s % 4]
                att_ps = attps.tile([W, O324], F32, tag="att")
                nc.tensor.matmul(
                    att_ps[:], xs[:, 1 : W + 1], wqkvT[:], start=True, stop=True
                )
                e2 = epool.tile([W, O324 * 2], BF16, tag="e", name=f"e{rep}_{s}")
                nc.scalar.activation(
                    e2[:].rearrange("p (o u) -> p o u", u=2),
                    att_ps[:][:, :, None].broadcast_to([W, O324, 2]),
                    AF.Exp,
                    scale=scale,
                )
                return e2

            def att_tail(rep, s, e2):
                # DVE: sum over q (free-axis reduce on the u=0 lane),
                # reciprocal (bf16, duplicated), attb in 2x mode.
                ev = e2[:].rearrange("p (g q u) -> p g q u", q=K2, u=2)
                evu = e2[:].rearrange("p (g q u) -> p u g q", q=K2, u=2)
                ssum = smpool.tile([W, 36], F32, tag="ssum")
                nc.vector.tensor_reduce(
                    ssum[:],
                    evu[:, 0:1],
                    axis=mybir.AxisListType.X,
                    op=mybir.AluOpType.add,
                )
                recip2 = smpool.tile([W, 36 * 2], BF16, tag="recip")
                with nc.allow_low_precision(reason="softmax recip bf16; tol 2e-2"):
                    nc.vector.reciprocal(
                        recip2[:].rearrange("p (g u) -> p g u", u=2),
                        ssum[:][:, :, None].broadcast_to([W, 36, 2]),
                    )
                if s in (1, ROWS - 2):
                    # image top/bottom: zero att rows outside the image
                    # (only these rows can be out of range on any core)
                    recipm2 = smpool.tile([W, 36 * 2], BF16, tag="recipm")
                    nc.vector.tensor_scalar_mul(
                        recipm2[:], recip2[:], masks[:, s : s + 1]
                    )
                    recip2 = recipm2
                attb = apool.tile([W, 36 * K2 * 2], BF16, tag="attb",
                                  name=f"attb{rep}_{s}")
                nc.vector.tensor_tensor(
                    attb[:].rearrange("p (g q u) -> p g q u", g=36, q=K2, u=2),
                    ev,
                    recip2[:]
                    .rearrange("p (g u) -> p g u", u=2)[:, :, None, :]
                    .broadcast_to([W, 36, K2, 2]),
                    op=mybir.AluOpType.mult,
                )
                attb_tiles[(rep, s)] = attb

            def products(rep, m):
                # prod[px, h, k, q, e, u] = att * V'.  DVE (bf16 2x mode,
                # 0.52 ns/elem) vs Pool (gpsimd TT, ~1.98 ns/elem): split so
                # both engines finish a row in ~5.2us.
                pt = ppool.tile([W, HEADS * K2 * K2 * HD], BF16, tag="prod",
                                name=f"prod{rep}_{m}")
                prod_tiles[(rep, m)] = pt
                vv = v_tiles[(rep, m)][:].rearrange(
                    "p (h q e u) -> p h q e u", h=HEADS, q=K2, e=HD // 2, u=2
                )
                av = attb_tiles[(rep, m)][:].rearrange(
                    "p (h k q u) -> p h k q u", h=HEADS, k=K2, q=K2, u=2
                )
                pv = pt[:].rearrange(
                    "p (h k q e u) -> p h k q e u",
                    h=HEADS, k=K2, q=K2, e=HD // 2, u=2,
                )

                def emit(eng, h, k0, k1):
                    kn = k1 - k0
                    a_b = av[:, h, k0:k1][:, :, :, None, :].broadcast_to(
                        [W, kn, K2, HD // 2, 2]
                    )
                    v_b = vv[:, h][:, None, :, :, :].broadcast_to(
                        [W, kn, K2, HD // 2, 2]
                    )
                    eng.tensor_tensor(
                        pv[:, h, k0:k1], a_b, v_b, op=mybir.AluOpType.mult
                    )

                # Unit = one (h, k) pair (288 elems). Pool takes the last
                # POOL_UNITS units; DVE the rest.
                dve_until = HEADS * K2 - POOL_UNITS
                for h in range(HEADS):
                    lo, hi = h * K2, h * K2 + K2
                    d_hi = min(hi, dve_until)
                    if d_hi > lo:
                        emit(nc.vector, h, 0, d_hi - lo)
                    p_lo = max(lo, dve_until)
                    if hi > p_lo:
                        emit(nc.gpsimd, h, p_lo - lo, K2)

            def fold_group(rep, i0, nrows):
                # fold + q-reduce: 27*(nrows+2) shift matmuls into one PSUM
                # region; first matmul is full-width (start=True covers all
                # row blocks)
                ypre_ps = ypreps.tile([W, nrows * C], F32, tag="ypre",
                                      name=f"ypre{rep}_{i0}")
                ts_ = sorted(
                    range(i0 - 1, i0 + nrows + 1),
                    key=lambda t_: -min(i0 + nrows - 1, t_ + 1) + max(i0, t_ - 1),
                )
                mms = []
                for t in ts_:
                    jlo = max(i0, t - 1)
                    jhi = min(i0 + nrows - 1, t + 1)
                    if jlo > jhi or not (1 <= t <= ROWS - 2):
                        continue
                    pv6 = prod_tiles[(rep, t)][:].rearrange(
                        "p (h a b q d) -> p a h b q d",
                        h=HEADS, a=K, b=K, q=K2, d=HD,
                    )
                    a0 = jlo - t + 1
                    alen = jhi - jlo + 1
                    for b1 in range(K):
                        for q in range(K2):
                            mms.append((t, jlo, a0, alen, b1, q, pv6))
                for n_, (t, jlo, a0, alen, b1, q, pv6) in enumerate(mms):
                    rhs = pv6[:, a0 : a0 + alen, :, b1, q, :]
                    nc.tensor.matmul(
                        ypre_ps[:, (jlo - i0) * C : (jlo - i0 + alen) * C],
                        shifts[:, b1 * W : (b1 + 1) * W],
                        rhs,
                        start=(n_ == 0),
                        stop=(n_ == len(mms) - 1),
                    )
                ypre_sb = ypool.tile([W, nrows * C], BF16, tag="ypre_sb",
                                     name=f"ypre_sb{rep}_{i0}")
                nc.scalar.copy(ypre_sb[:], ypre_ps[:])
                ysb_tiles[(rep, i0)] = (ypre_sb, nrows)

            def yt_transposes(rep, i0):
                ypre_sb, nrows = ysb_tiles[(rep, i0)]
                yts = []
                for r_ in range(nrows):
                    yt_ps = transps.tile([C, W], BF16, tag="tr")
                    nc.tensor.transpose(
                        yt_ps[:], ypre_sb[:, r_ * C : (r_ + 1) * C], eye_bf
                    )
                    yt_sb = ytpool.tile([C, W], BF16, tag="yt_sb")
                    nc.scalar.copy(yt_sb[:], yt_ps[:])
                    yts.append(yt_sb)
                ytsb_tiles[(rep, i0)] = yts

            def proj_group(rep, i0):
                yts = ytsb_tiles.pop((rep, i0))
                nrows = len(yts)
                fin_ps = finps.tile([W, nrows * C], F32, tag="fin",
                                    name=f"fin{rep}_{i0}")
                for r_, yt_sb in enumerate(yts):
                    nc.tensor.matmul(
                        fin_ps[:, r_ * C : (r_ + 1) * C],
                        yt_sb[:],
                        wprojT[:],
                        start=True,
                        stop=True,
                    )
                fin_sb = fpool.tile([W, nrows * C], F32, tag="fin_sb",
                                    name=f"fin_sb{rep}_{i0}")
                nc.scalar.copy(fin_sb[:], fin_ps[:])
                dst = out_d[i0 - 2 : i0 - 2 + nrows]
                nc.sync.dma_start(
                    out=dst.rearrange("r w c -> w r c"),
                    in_=fin_sb[:].rearrange("w (r c) -> w r c", r=nrows),
                )

            def emit_step(rep, v):
                # one virtual pipeline step of rep: v in [-1, ROWS+6]
                if v == -1:
                    # prologue: prefetch + first transpose
                    load_x(rep, 0)
                    load_x(rep, 1)
                    transpose_x(0, cast_x(rep, 0))
                    return
                if v + 2 < ROWS:
                    load_x(rep, v + 2)
                xb_next = cast_x(rep, v + 1) if v + 1 < ROWS else None
                # PE: att + U first (inputs one row old)
                e2_row = None
                if 1 <= v <= ROWS - 2:
                    e2_row = att_row(rep, v)
                if v < ROWS:
                    u_matmuls(rep, v)
                # PE: lagged y-transposes (before x-transpose: transps PSUM
                # slots rotate yt0,yt1,yt2,xT with prompt ACT evacuations)
                g = v - 6
                fold_due = v >= 8 and (v - 8) % 3 == 0
                if fold_due and 5 <= g and g - 3 <= ROWS - 3:
                    yt_transposes(rep, g - 3)
                if xb_next is not None:
                    transpose_x(v + 1, xb_next)
                # DVE/Pool: products for mult-row m = v - 2
                m = v - 2
                if 1 <= m <= ROWS - 2:
                    products(rep, m)
                # PE: this row's fold, then lagged projection
                if fold_due and g <= ROWS - 3:
                    fold_group(rep, g, 3 if g <= ROWS - 6 else 1)
                if fold_due and 5 <= g and g - 3 <= ROWS - 3:
                    proj_group(rep, g - 3)
                # softmax tail (after products in DVE/Pool queues)
                if e2_row is not None:
                    att_tail(rep, v, e2_row)

            for S in range(-1, (repeat - 1) * ROWS + ROWS + 7):
                for rep in range(repeat):
                    v = S - rep * ROWS
                    if -1 <= v <= ROWS + 6:
                        emit_step(rep, v)

    _dedup_ldweights(nc)
    _split_multi_waits(nc)
    return nc


def _dedup_ldweights(nc):
    """Delete InstLdweights whose weights AP is identical to the previous
    weight load on the PE stream (weights persist in the array). Transposes
    load their own stationary, so they invalidate the tracked state. Waits on
    a deleted LDW move to the next kept instruction."""
    import concourse.mybir as mb

    def apkey(arg):
        t = getattr(arg, "bass_ap", None)
        if t is None:
            return str(arg)
        return (t.tensor.name, t.offset, tuple(map(tuple, t.ap)))

    for f in nc.m.functions:
        for bb in f.blocks:
            last_key = None
            pending_waits = []
            out = []
            for inst in bb.instructions:
                eng = str(getattr(inst, "engine", ""))
                tname = type(inst).__name__
                if not eng.endswith("PE"):
                    out.append(inst)
                    continue
                if tname == "InstLdweights":
                    key = tuple(apkey(a) for a in inst.ins)
                    if key == last_key:
                        si = inst.sync_info
                        if si is not None and si.on_wait:
                            pending_waits.extend(si.on_wait)
                        continue
                    last_key = key
                elif tname == "InstMatmult":
                    if getattr(inst, "is_transpose", False):
                        last_key = None
                else:
                    last_key = None
                if pending_waits:
                    si = inst.sync_info
                    if si is None:
                        inst.sync_info = mb.SyncInfo(
                            on_wait=list(pending_waits), on_update=[]
                        )
                    else:
                        si.on_wait = list(pending_waits) + list(si.on_wait)
                    pending_waits = []
                out.append(inst)
            assert not pending_waits
            bb.instructions[:] = out


def _split_multi_waits(nc, limit=1):
    """Walrus codegen accepts at most one sync-wait per instruction on some
    engine structs. Split extras into same-engine NoOps preceding the
    instruction (in-order queues make sequential waits equivalent)."""
    nid = [0]

    def mknop(inst, wait):
        nid[0] += 1
        return mybir.InstNoOp(
            name=f"I-waitnop-{nid[0]}",
            engine=inst.engine,
            ins=[],
            outs=[],
            sync_info=mybir.SyncInfo(on_wait=[wait], on_update=[]),
        )

    for f in nc.m.functions:
        for bb in f.blocks:
            out = []
            for inst in bb.instructions:
                si = inst.sync_info
                if si is not None and si.on_wait and len(si.on_wait) > limit:
                    waits = list(si.on_wait)
                    for w in waits[:-limit]:
                        out.append(mknop(inst, w))
                    si.on_wait = waits[-limit:]
                out.append(inst)
            bb.instructions[:] = out


def prep_inputs(x, w_qkv, w_v, w_proj):
    """Host-side input prep -> per-core input maps."""
    wqkvT = np.ascontiguousarray(w_qkv.T).astype(np.float32)  # [C, 324]
    # wvT[j, q*C + c] = w_v[q, c, j]
    wvT = np.ascontiguousarray(
        np.transpose(w_v, (2, 0, 1)).reshape(C, K2 * C)
    ).astype(np.float32)
    wprojT = np.ascontiguousarray(w_proj.T).astype(np.float32)  # [c, o]
    # S_b[n', j] = delta(n' == j - b + 1) = eye(k = b - 1)
    shifts = np.concatenate(
        [np.eye(W, k=b - 1, dtype=np.float32) for b in range(3)], axis=1
    )

    in_maps = []
    for core in range(N_CORES):
        bb = core // 2
        half = core % 2
        r0 = half * (H // 2)
        # rows r0-2 .. r0+65 with zero pad outside image
        xs = np.zeros((ROWS, W, C), np.float32)
        lo = max(0, r0 - 2)
        hi = min(H, r0 + H // 2 + 2)
        xs[lo - (r0 - 2) : hi - (r0 - 2)] = x[bb, lo:hi]
        # mask: shard row s = image row r0 - 2 + s ; valid iff 0 <= row < H
        mk = np.zeros((ROWS,), np.float32)
        rows = r0 - 2 + np.arange(ROWS)
        mk[(rows >= 0) & (rows < H)] = 1.0
        masks = np.ascontiguousarray(np.broadcast_to(mk[None, :], (W, ROWS)))
        in_maps.append(
            {
                "x": xs,
                "wqkvT": wqkvT,
                "wvT": wvT,
                "wprojT": wprojT,
                "shifts": shifts,
                "masks": masks,
            }
        )
    return in_maps


def kernel(x, w_qkv, w_v, w_proj, _trace=False):
    global LAST_RESULTS
    if "nc" not in _CACHE:
        _CACHE["nc"] = build_graph()
    nc = _CACHE["nc"]
    in_maps = prep_inputs(
        np.asarray(x, np.float32),
        np.asarray(w_qkv, np.float32),
        np.asarray(w_v, np.float32),
        np.asarray(w_proj, np.float32),
    )
    res = run_bass_kernel_spmd(nc, in_maps, list(range(N_CORES)), trace=_trace)
    LAST_RESULTS = res
    y = np.zeros((B, H, W, C), np.float32)
    for core in range(N_CORES):
        bb = core // 2
        half = core % 2
        r0 = half * (H // 2)
        y[bb, r0 : r0 + H // 2] = res.results[core]["out"]
    return y



# revision 22
# speedup vs baseline: 998.8631x; 1.0449x over previous
"""Trainium2 Bass kernel for CSA (3x3 convolutional self-attention).

Reference computation (per sample):
  att = softmax over q of (x @ w_qkv.T) / sqrt(hd), per (head, p)    [N, heads, 9, 9]
  U_q = shifted(x) @ w_v[q].T  (q = 3x3 window position)             [N, C] per q
  out[n, p, c] = sum_q att[n, h(c), p, q] * U_q[n + off_q, c]
  y_pre[m, c]  = sum_p out[m - off_p, p, c]    (fold)
  y = y_pre @ w_proj.T

Distribution: 8 cores = 4 samples x 2 row-halves (64 rows each + 2-row halo).

Per-core software pipeline over source rows s (68 = 64 + 2*2 halo), with
engine balance (steady-state ns/row, TimelineSim cost model):
  PE  (~5.0us): att matmul; 9 U matmuls; x-row transpose (bf16); fused
      fold+q-reduce (81 shift-matrix matmuls/3 rows into PSUM); y transposes
      + projection, lagged one fold group so PE never waits on ACT.
  DVE (~5.2us): softmax q-sum (free-axis reduce); reciprocal (bf16, written
      duplicated); attb normalize-broadcast (2x mode thanks to the
      duplicated exp); products for heads 0-2 (bf16 2x mode, 0.52 ns/elem),
      for mult-row m = s-2 (2 rows of slack).
  Pool(~5.3us): products for head 3 (gpsimd TT, ~1.98 ns/elem at the 0.42
      Q7 software efficiency).
  ACT (~3.5us): x bf16 cast; exp (written bf16, duplicated u=2 so all DVE
      consumers see packed [1,2] innermost dims); PSUM evacuations.
The fold for group g fires 6 rows after its first output row so all product
tiles are at least one row old (PE streams without stalls and holds its
ramped 2.4 GHz p-state). Reps are software-pipelined into one flat stream:
rep r's PE-only fold/proj tail (virtual rows 68..74) overlaps rep r+1's
DVE/Pool ramp, making the marginal per-rep cost ~ the steady-state floor.
Image-edge correctness is data-driven via per-row masks (single SPMD graph).
"""

import sys

sys.path.insert(0, "/opt/trn_rl_repo")

import numpy as np

import concourse.bass as bass
import concourse.mybir as mybir
import concourse.tile as tile
from concourse.bass_utils import run_bass_kernel_spmd

F32 = mybir.dt.float32
BF16 = mybir.dt.bfloat16
AF = mybir.ActivationFunctionType

K = 3
K2 = 9
HEADS = 4
C = 128
HD = 32
B, H, W = 4, 128, 128
ROWS = H // 2 + 4  # 68 rows per shard (64 + 2 halo each side)
N_CORES = 8
O324 = K2 * K2 * HEADS  # 324
POOL_UNITS = 9  # (h,k) product units (288 elems each) assigned to Pool,
                # counting from the end (h3 k8 backwards)

_CACHE = {}
LAST_RESULTS = None  # test harness can inspect exec_time


def build_graph(repeat=1):
    nc = bass.Bass()

    x_d = nc.declare_dram_parameter("x", [ROWS, W, C], F32, isOutput=False)
    wqkvT_d = nc.declare_dram_parameter("wqkvT", [C, O324], F32, isOutput=False)
    wvT_d = nc.declare_dram_parameter("wvT", [C, K2 * C], F32, isOutput=False)
    wprojT_d = nc.declare_dram_parameter("wprojT", [C, C], F32, isOutput=False)
    shifts_d = nc.declare_dram_parameter("shifts", [W, 3 * W], F32, isOutput=False)
    masks_d = nc.declare_dram_parameter("masks", [W, ROWS], F32, isOutput=False)
    out_d = nc.declare_dram_parameter("out", [H // 2, W, C], F32, isOutput=True)

    from contextlib import ExitStack
    with tile.TileContext(nc) as tc, ExitStack() as es:
        cpool = es.enter_context(tc.tile_pool(name="const", bufs=1))
        spool = es.enter_context(tc.tile_pool(name="stage", bufs=1))
        xpool = es.enter_context(tc.tile_pool(name="xin", bufs=4))
        xbpool = es.enter_context(tc.tile_pool(name="xbf", bufs=3))
        epool = es.enter_context(tc.tile_pool(name="esb", bufs=3))
        smpool = es.enter_context(tc.tile_pool(name="small", bufs=8))
        apool = es.enter_context(tc.tile_pool(name="attb", bufs=4))
        vpool = es.enter_context(tc.tile_pool(name="vprime", bufs=6))
        ppool = es.enter_context(tc.tile_pool(name="prod", bufs=7))
        ypool = es.enter_context(tc.tile_pool(name="ysb", bufs=2))
        ytpool = es.enter_context(tc.tile_pool(name="ytsb", bufs=4))
        fpool = es.enter_context(tc.tile_pool(name="fsb", bufs=2))
        transps = es.enter_context(tc.tile_pool(name="tps", bufs=2, space="PSUM"))
        attps = es.enter_context(tc.tile_pool(name="attps", bufs=1, space="PSUM"))
        ups = es.enter_context(tc.tile_pool(name="ups", bufs=3, space="PSUM"))
        ypreps = es.enter_context(tc.tile_pool(name="ypreps", bufs=1, space="PSUM"))
        finps = es.enter_context(tc.tile_pool(name="finps", bufs=1, space="PSUM"))

        # ---- constants: DMA f32, cast to bf16 where needed.  Ordered so the
        # pipeline can start ASAP: shifts (eye for the first transpose) and
        # the first x rows go before the big weight tensors. ----
        def load_const_bf16(dram_ap, shape, name):
            st = spool.tile(shape, F32, tag=f"stage_{name}", name=f"stage_{name}")
            nc.sync.dma_start(out=st[:], in_=dram_ap)
            t = cpool.tile(shape, BF16, tag=name, name=name)
            nc.vector.tensor_copy(t[:], st[:])
            return t

        shifts = load_const_bf16(shifts_d[:], [W, 3 * W], "shifts")
        wqkvT = load_const_bf16(wqkvT_d[:], [C, O324], "wqkvT")
        wvT = load_const_bf16(wvT_d[:], [C, K2 * C], "wvT")
        wprojT = load_const_bf16(wprojT_d[:], [C, C], "wprojT")
        masks = cpool.tile([W, ROWS], F32, tag="masks")
        nc.sync.dma_start(out=masks[:], in_=masks_d[:])

        eye_bf = shifts[:, W : 2 * W]  # shift b=1 is the identity


        # persistent x-transpose tiles (manual rotation; edge columns are
        # zeroed once and never rewritten -> image border padding)
        xtp = [
            cpool.tile([C, W + 2], BF16, tag=f"xtp{i}", name=f"xtp{i}")
            for i in range(4)
        ]
        for i in range(4):
            nc.gpsimd.memset(xtp[i][:, 0:1], 0.0)
            nc.gpsimd.memset(xtp[i][:, W + 1 : W + 2], 0.0)

        scale = float(HD) ** -0.5

        # ---- flattened software pipeline across reps: rep r's virtual row
        # v = S - r*ROWS runs from -1 (prefetch) to ROWS+6 (tail folds), so
        # rep r's PE-only fold/proj tail overlaps rep r+1's DVE/Pool ramp ----
        x_tiles = {}
        v_tiles = {}
        prod_tiles = {}
        attb_tiles = {}
        ysb_tiles = {}
        ytsb_tiles = {}

        if True:  # keep indentation of the original rep-loop body
            def load_x(rep, s):
                x_sb = xpool.tile([W, C], F32, tag="x", name=f"x{rep}_{s}")
                nc.sync.dma_start(out=x_sb[:], in_=x_d[s])
                x_tiles[(rep, s)] = x_sb

            def cast_x(rep, s):
                xb = xbpool.tile([W, C], BF16, tag="xb", name=f"xb{rep}_{s}")
                nc.scalar.copy(xb[:], x_tiles.pop((rep, s))[:])
                return xb

            def transpose_x(s, xb):
                xt_ps = transps.tile([C, W], BF16, tag="tr")
                nc.tensor.transpose(xt_ps[:], xb[:], eye_bf)
                nc.scalar.copy(xtp[s % 4][:, 1 : W + 1], xt_ps[:])

            def get_vtile(rep, t):
                if (rep, t) not in v_tiles:
                    v_tiles[(rep, t)] = vpool.tile(
                        [W, HEADS * K2 * HD], BF16, tag="vp", name=f"vp{rep}_{t}"
                    )
                return v_tiles[(rep, t)]

            def u_matmuls(rep, s):
                # q = a*3 + b ; contributes to mult-row t = s - a + 1
                # One PSUM tile per a (3 q's) so each evacuates in one ACT op.
                xs = xtp[s % 4]
                u_ts = [
                    ups.tile([W, 3 * C], F32, tag="u", name=f"u{rep}_{s}_{a_}")
                    for a_ in range(K)
                ]
                for b in (1, 0, 2):
                    for a in range(K):
                        t = s - a + 1
                        if not (1 <= t <= ROWS - 2):
                            continue
                        q = a * K + b
                        nc.tensor.matmul(
                            u_ts[a][:, b * C : (b + 1) * C],
                            xs[:, b : b + W],
                            wvT[:, q * C : (q + 1) * C],
                            start=True,
                            stop=True,
                        )
                for a in range(K):
                    t = s - a + 1
                    if not (1 <= t <= ROWS - 2):
                        continue
                    vt = get_vtile(rep, t)
                    vdst = vt[:].rearrange(
                        "p (h q d) -> p h q d", h=HEADS, q=K2, d=HD
                    )[:, :, 3 * a : 3 * a + 3, :]
                    usrc = u_ts[a][:].rearrange(
                        "p (q h d) -> p h q d", q=K, h=HEADS, d=HD
                    )
                    nc.scalar.copy(vdst, usrc)

            def att_row(rep, s):
                # PE scores -> ACT exp, written bf16 and duplicated (u=2) so
                # every downstream DVE op sees packed [1,2] innermost dims and
                # runs in 2x mode. (softmax tail is emitted in att_tail AFTER
                # the row's products so DVE/Pool queue heads never idle-wait
                # on same-row exp)
                xs = xtp[s % 4]
                att_ps = attps.tile([W, O324], F32, tag="att")
                nc.tensor.matmul(
                    att_ps[:], xs[:, 1 : W + 1], wqkvT[:], start=True, stop=True
                )
                e2 = epool.tile([W, O324 * 2], BF16, tag="e", name=f"e{rep}_{s}")
                nc.scalar.activation(
                    e2[:].rearrange("p (o u) -> p o u", u=2),
                    att_ps[:][:, :, None].broadcast_to([W, O324, 2]),
                    AF.Exp,
                    scale=scale,
                )
                return e2

            def att_tail(rep, s, e2):
                # DVE: sum over q (free-axis reduce on the u=0 lane),
                # reciprocal (bf16, duplicated), attb in 2x mode.
                ev = e2[:].rearrange("p (g q u) -> p g q u", q=K2, u=2)
                evu = e2[:].rearrange("p (g q u) -> p u g q", q=K2, u=2)
                ssum = smpool.tile([W, 36], F32, tag="ssum")
                nc.vector.tensor_reduce(
                    ssum[:],
                    evu[:, 0:1],
                    axis=mybir.AxisListType.X,
                    op=mybir.AluOpType.add,
                )
                recip2 = smpool.tile([W, 36 * 2], BF16, tag="recip")
                with nc.allow_low_precision(reason="softmax recip bf16; tol 2e-2"):
                    nc.vector.reciprocal(
                        recip2[:].rearrange("p (g u) -> p g u", u=2),
                        ssum[:][:, :, None].broadcast_to([W, 36, 2]),
                    )
                if s in (1, ROWS - 2):
                    # image top/bottom: zero att rows outside the image
                    # (only these rows can be out of range on any core)
                    recipm2 = smpool.tile([W, 36 * 2], BF16, tag="recipm")
                    nc.vector.tensor_scalar_mul(
                        recipm2[:], recip2[:], masks[:, s : s + 1]
                    )
                    recip2 = recipm2
                attb = apool.tile([W, 36 * K2 * 2], BF16, tag="attb",
                                  name=f"attb{rep}_{s}")
                nc.vector.tensor_tensor(
                    attb[:].rearrange("p (g q u) -> p g q u", g=36, q=K2, u=2),
                    ev,
                    recip2[:]
                    .rearrange("p (g u) -> p g u", u=2)[:, :, None, :]
                    .broadcast_to([W, 36, K2, 2]),
                    op=mybir.AluOpType.mult,
                )
                attb_tiles[(rep, s)] = attb

            def products(rep, m):
                # prod[px, h, k, q, e, u] = att * V'.  DVE (bf16 2x mode,
                # 0.52 ns/elem) vs Pool (gpsimd TT, ~1.98 ns/elem): split so
                # both engines finish a row in ~5.2us.
                pt = ppool.tile([W, HEADS * K2 * K2 * HD], BF16, tag="prod",
                                name=f"prod{rep}_{m}")
                prod_tiles[(rep, m)] = pt
                vv = v_tiles[(rep, m)][:].rearrange(
                    "p (h q e u) -> p h q e u", h=HEADS, q=K2, e=HD // 2, u=2
                )
                av = attb_tiles[(rep, m)][:].rearrange(
                    "p (h k q u) -> p h k q u", h=HEADS, k=K2, q=K2, u=2
                )
                pv = pt[:].rearrange(
                    "p (h k q e u) -> p h k q e u",
                    h=HEADS, k=K2, q=K2, e=HD // 2, u=2,
                )

                def emit(eng, h, k0, k1):
                    kn = k1 - k0
                    a_b = av[:, h, k0:k1][:, :, :, None, :].broadcast_to(
                        [W, kn, K2, HD // 2, 2]
                    )
                    v_b = vv[:, h][:, None, :, :, :].broadcast_to(
                        [W, kn, K2, HD // 2, 2]
                    )
                    eng.tensor_tensor(
                        pv[:, h, k0:k1], a_b, v_b, op=mybir.AluOpType.mult
                    )

                # Unit = one (h, k) pair (288 elems). Pool takes the last
                # POOL_UNITS units; DVE the rest.
                dve_until = HEADS * K2 - POOL_UNITS
                for h in range(HEADS):
                    lo, hi = h * K2, h * K2 + K2
                    d_hi = min(hi, dve_until)
                    if d_hi > lo:
                        emit(nc.vector, h, 0, d_hi - lo)
                    p_lo = max(lo, dve_until)
                    if hi > p_lo:
                        emit(nc.gpsimd, h, p_lo - lo, K2)

            def fold_group(rep, i0, nrows):
                # fold + q-reduce: 27*(nrows+2) shift matmuls into one PSUM
                # region; first matmul is full-width (start=True covers all
                # row blocks)
                ypre_ps = ypreps.tile([W, nrows * C], F32, tag="ypre",
                                      name=f"ypre{rep}_{i0}")
                ts_ = sorted(
                    range(i0 - 1, i0 + nrows + 1),
                    key=lambda t_: -min(i0 + nrows - 1, t_ + 1) + max(i0, t_ - 1),
                )
                tinfos = []
                for t in ts_:
                    jlo = max(i0, t - 1)
                    jhi = min(i0 + nrows - 1, t + 1)
                    if jlo > jhi or not (1 <= t <= ROWS - 2):
                        continue
                    pv6 = prod_tiles[(rep, t)][:].rearrange(
                        "p (h a b q d) -> p a h b q d",
                        h=HEADS, a=K, b=K, q=K2, d=HD,
                    )
                    tinfos.append((t, jlo, jlo - t + 1, jhi - jlo + 1, pv6))
                # t-major (so PE pipelines against products still finishing
                # on Pool for the newest t), with a palindromic b order so
                # consecutive t-blocks share their boundary shift stationary
                # (LDW dedup: 15 -> 11 loads per group; LDW is free in the
                # cost model but ~107ns each on real HW)
                mms = []
                for ti, (t, jlo, a0, alen, pv6) in enumerate(tinfos):
                    border = (0, 1, 2) if ti % 2 == 0 else (2, 1, 0)
                    for b1 in border:
                        for q in range(K2):
                            mms.append((t, jlo, a0, alen, b1, q, pv6))
                for n_, (t, jlo, a0, alen, b1, q, pv6) in enumerate(mms):
                    rhs = pv6[:, a0 : a0 + alen, :, b1, q, :]
                    nc.tensor.matmul(
                        ypre_ps[:, (jlo - i0) * C : (jlo - i0 + alen) * C],
                        shifts[:, b1 * W : (b1 + 1) * W],
                        rhs,
                        start=(n_ == 0),
                        stop=(n_ == len(mms) - 1),
                    )
                ypre_sb = ypool.tile([W, nrows * C], BF16, tag="ypre_sb",
                                     name=f"ypre_sb{rep}_{i0}")
                nc.scalar.copy(ypre_sb[:], ypre_ps[:])
                ysb_tiles[(rep, i0)] = (ypre_sb, nrows)

            def yt_transposes(rep, i0):
                ypre_sb, nrows = ysb_tiles[(rep, i0)]
                yts = []
                for r_ in range(nrows):
                    yt_ps = transps.tile([C, W], BF16, tag="tr")
                    nc.tensor.transpose(
                        yt_ps[:], ypre_sb[:, r_ * C : (r_ + 1) * C], eye_bf
                    )
                    yt_sb = ytpool.tile([C, W], BF16, tag="yt_sb")
                    nc.scalar.copy(yt_sb[:], yt_ps[:])
                    yts.append(yt_sb)
                ytsb_tiles[(rep, i0)] = yts

            def proj_group(rep, i0):
                yts = ytsb_tiles.pop((rep, i0))
                nrows = len(yts)
                fin_ps = finps.tile([W, nrows * C], F32, tag="fin",
                                    name=f"fin{rep}_{i0}")
                for r_, yt_sb in enumerate(yts):
                    nc.tensor.matmul(
                        fin_ps[:, r_ * C : (r_ + 1) * C],
                        yt_sb[:],
                        wprojT[:],
                        start=True,
                        stop=True,
                    )
                fin_sb = fpool.tile([W, nrows * C], F32, tag="fin_sb",
                                    name=f"fin_sb{rep}_{i0}")
                nc.scalar.copy(fin_sb[:], fin_ps[:])
                dst = out_d[i0 - 2 : i0 - 2 + nrows]
                nc.sync.dma_start(
                    out=dst.rearrange("r w c -> w r c"),
                    in_=fin_sb[:].rearrange("w (r c) -> w r c", r=nrows),
                )

            def emit_step(rep, v):
                # one virtual pipeline step of rep: v in [-1, ROWS+6]
                if v == -1:
                    # prologue: prefetch + first transpose
                    load_x(rep, 0)
                    load_x(rep, 1)
                    transpose_x(0, cast_x(rep, 0))
                    return
                if v + 2 < ROWS:
                    load_x(rep, v + 2)
                xb_next = cast_x(rep, v + 1) if v + 1 < ROWS else None
                # PE: att + U first (inputs one row old)
                e2_row = None
                if 1 <= v <= ROWS - 2:
                    e2_row = att_row(rep, v)
                if v < ROWS:
                    u_matmuls(rep, v)
                # PE: lagged y-transposes (before x-transpose: transps PSUM
                # slots rotate yt0,yt1,yt2,xT with prompt ACT evacuations)
                g = v - 6
                fold_due = v >= 8 and (v - 8) % 3 == 0
                if fold_due and 5 <= g and g - 3 <= ROWS - 3:
                    yt_transposes(rep, g - 3)
                if xb_next is not None:
                    transpose_x(v + 1, xb_next)
                # DVE/Pool: products for mult-row m = v - 2
                m = v - 2
                if 1 <= m <= ROWS - 2:
                    products(rep, m)
                # PE: this row's fold, then lagged projection
                if fold_due and g <= ROWS - 3:
                    fold_group(rep, g, 3 if g <= ROWS - 6 else 1)
                if fold_due and 5 <= g and g - 3 <= ROWS - 3:
                    proj_group(rep, g - 3)
                # softmax tail (after products in DVE/Pool queues)
                if e2_row is not None:
                    att_tail(rep, v, e2_row)

            for S in range(-1, (repeat - 1) * ROWS + ROWS + 7):
                for rep in range(repeat):
                    v = S - rep * ROWS
                    if -1 <= v <= ROWS + 6:
                        emit_step(rep, v)

    _dedup_ldweights(nc)
    _split_multi_waits(nc)
    return nc


def _dedup_ldweights(nc):
    """Delete InstLdweights whose weights AP is identical to the previous
    weight load on the PE stream (weights persist in the array). Transposes
    load their own stationary, so they invalidate the tracked state. Waits on
    a deleted LDW move to the next kept instruction."""
    import concourse.mybir as mb

    def apkey(arg):
        t = getattr(arg, "bass_ap", None)
        if t is None:
            return str(arg)
        return (t.tensor.name, t.offset, tuple(map(tuple, t.ap)))

    for f in nc.m.functions:
        for bb in f.blocks:
            last_key = None
            pending_waits = []
            out = []
            for inst in bb.instructions:
                eng = str(getattr(inst, "engine", ""))
                tname = type(inst).__name__
                if not eng.endswith("PE"):
                    out.append(inst)
                    continue
                if tname == "InstLdweights":
                    key = tuple(apkey(a) for a in inst.ins)
                    if key == last_key:
                        si = inst.sync_info
                        if si is not None and si.on_wait:
                            pending_waits.extend(si.on_wait)
                        continue
                    last_key = key
                elif tname == "InstMatmult":
                    if getattr(inst, "is_transpose", False):
                        last_key = None
                else:
                    last_key = None
                if pending_waits:
                    si = inst.sync_info
                    if si is None:
                        inst.sync_info = mb.SyncInfo(
                            on_wait=list(pending_waits), on_update=[]
                        )
                    else:
                        si.on_wait = list(pending_waits) + list(si.on_wait)
                    pending_waits = []
                out.append(inst)
            assert not pending_waits
            bb.instructions[:] = out


def _split_multi_waits(nc, limit=1):
    """Walrus codegen accepts at most one sync-wait per instruction on some
    engine structs. Split extras into same-engine NoOps preceding the
    instruction (in-order queues make sequential waits equivalent)."""
    nid = [0]

    def mknop(inst, wait):
        nid[0] += 1
        return mybir.InstNoOp(
            name=f"I-waitnop-{nid[0]}",
            engine=inst.engine,
            ins=[],
            outs=[],
            sync_info=mybir.SyncInfo(on_wait=[wait], on_update=[]),
        )

    for f in nc.m.functions:
        for bb in f.blocks:
            out = []
            for inst in bb.instructions:
                si = inst.sync_info
                if si is not None and si.on_wait and len(si.on_wait) > limit:
                    waits = list(si.on_wait)
                    for w in waits[:-limit]:
                        out.append(mknop(inst, w))
                    si.on_wait = waits[-limit:]
                out.append(inst)
            bb.instructions[:] = out


def prep_inputs(x, w_qkv, w_v, w_proj):
    """Host-side input prep -> per-core input maps."""
    wqkvT = np.ascontiguousarray(w_qkv.T).astype(np.float32)  # [C, 324]
    # wvT[j, q*C + c] = w_v[q, c, j]
    wvT = np.ascontiguousarray(
        np.transpose(w_v, (2, 0, 1)).reshape(C, K2 * C)
    ).astype(np.float32)
    wprojT = np.ascontiguousarray(w_proj.T).astype(np.float32)  # [c, o]
    # S_b[n', j] = delta(n' == j - b + 1) = eye(k = b - 1)
    shifts = np.concatenate(
        [np.eye(W, k=b - 1, dtype=np.float32) for b in range(3)], axis=1
    )

    in_maps = []
    for core in range(N_CORES):
        bb = core // 2
        half = core % 2
        r0 = half * (H // 2)
        # rows r0-2 .. r0+65 with zero pad outside image
        xs = np.zeros((ROWS, W, C), np.float32)
        lo = max(0, r0 - 2)
        hi = min(H, r0 + H // 2 + 2)
        xs[lo - (r0 - 2) : hi - (r0 - 2)] = x[bb, lo:hi]
        # mask: shard row s = image row r0 - 2 + s ; valid iff 0 <= row < H
        mk = np.zeros((ROWS,), np.float32)
        rows = r0 - 2 + np.arange(ROWS)
        mk[(rows >= 0) & (rows < H)] = 1.0
        masks = np.ascontiguousarray(np.broadcast_to(mk[None, :], (W, ROWS)))
        in_maps.append(
            {
                "x": xs,
                "wqkvT": wqkvT,
                "wvT": wvT,
                "wprojT": wprojT,
                "shifts": shifts,
                "masks": masks,
            }
        )
    return in_maps


def kernel(x, w_qkv, w_v, w_proj, _trace=False):
    global LAST_RESULTS
    if "nc" not in _CACHE:
        _CACHE["nc"] = build_graph()
    nc = _CACHE["nc"]
    in_maps = prep_inputs(
        np.asarray(x, np.float32),
        np.asarray(w_qkv, np.float32),
        np.asarray(w_v, np.float32),
        np.asarray(w_proj, np.float32),
    )
    res = run_bass_kernel_spmd(nc, in_maps, list(range(N_CORES)), trace=_trace)
    LAST_RESULTS = res
    y = np.zeros((B, H, W, C), np.float32)
    for core in range(N_CORES):
        bb = core // 2
        half = core % 2
        r0 = half * (H // 2)
        y[bb, r0 : r0 + H // 2] = res.results[core]["out"]
    return y



# revision 23
# speedup vs baseline: 1063.2052x; 1.0644x over previous
"""Trainium2 Bass kernel for CSA (3x3 convolutional self-attention).

Reference computation (per sample):
  att = softmax over q of (x @ w_qkv.T) / sqrt(hd), per (head, p)    [N, heads, 9, 9]
  U_q = shifted(x) @ w_v[q].T  (q = 3x3 window position)             [N, C] per q
  out[n, p, c] = sum_q att[n, h(c), p, q] * U_q[n + off_q, c]
  y_pre[m, c]  = sum_p out[m - off_p, p, c]    (fold)
  y = y_pre @ w_proj.T

Distribution: 8 cores = 4 samples x 2 row-halves (64 rows each + 2-row halo).

Per-core software pipeline over source rows s (68 = 64 + 2*2 halo), with
engine balance (steady-state ns/row, TimelineSim cost model):
  PE  (~5.0us): att matmul; 9 U matmuls; x-row transpose (bf16); fused
      fold+q-reduce (81 shift-matrix matmuls/3 rows into PSUM); y transposes
      + projection, lagged one fold group so PE never waits on ACT.
  DVE (~5.2us): softmax q-sum (free-axis reduce); reciprocal (bf16, written
      duplicated); attb normalize-broadcast (2x mode thanks to the
      duplicated exp); products for heads 0-2 (bf16 2x mode, 0.52 ns/elem),
      for mult-row m = s-2 (2 rows of slack).
  Pool(~5.3us): products for head 3 (gpsimd TT, ~1.98 ns/elem at the 0.42
      Q7 software efficiency).
  ACT (~3.5us): x bf16 cast; exp (written bf16, duplicated u=2 so all DVE
      consumers see packed [1,2] innermost dims); PSUM evacuations.
The fold for group g fires 6 rows after its first output row so all product
tiles are at least one row old (PE streams without stalls and holds its
ramped 2.4 GHz p-state). Reps are software-pipelined into one flat stream:
rep r's PE-only fold/proj tail (virtual rows 68..74) overlaps rep r+1's
DVE/Pool ramp, making the marginal per-rep cost ~ the steady-state floor.
Image-edge correctness is data-driven via per-row masks (single SPMD graph).
"""

import sys

sys.path.insert(0, "/opt/trn_rl_repo")

import numpy as np

import concourse.bass as bass
import concourse.mybir as mybir
import concourse.tile as tile
from concourse.bass_utils import run_bass_kernel_spmd

F32 = mybir.dt.float32
BF16 = mybir.dt.bfloat16
AF = mybir.ActivationFunctionType

K = 3
K2 = 9
HEADS = 4
C = 128
HD = 32
B, H, W = 4, 128, 128
ROWS = H // 2 + 4  # 68 rows per shard (64 + 2 halo each side)
N_CORES = 8
O324 = K2 * K2 * HEADS  # 324
POOL_UNITS = 9  # (h,k) product units (288 elems each) assigned to Pool,
                # counting from the end (h3 k8 backwards)

_CACHE = {}
LAST_RESULTS = None  # test harness can inspect exec_time


def build_graph(repeat=1):
    nc = bass.Bass()

    x_d = nc.declare_dram_parameter("x", [ROWS, W, C], F32, isOutput=False)
    wqkvT_d = nc.declare_dram_parameter("wqkvT", [C, O324], F32, isOutput=False)
    wvT_d = nc.declare_dram_parameter("wvT", [C, K2 * C], F32, isOutput=False)
    wprojT_d = nc.declare_dram_parameter("wprojT", [C, C], F32, isOutput=False)
    shifts_d = nc.declare_dram_parameter("shifts", [W, 3 * W], F32, isOutput=False)
    masks_d = nc.declare_dram_parameter("masks", [W, ROWS], F32, isOutput=False)
    out_d = nc.declare_dram_parameter("out", [H // 2, W, C], F32, isOutput=True)

    from contextlib import ExitStack
    with tile.TileContext(nc) as tc, ExitStack() as es:
        cpool = es.enter_context(tc.tile_pool(name="const", bufs=1))
        spool = es.enter_context(tc.tile_pool(name="stage", bufs=1))
        xpool = es.enter_context(tc.tile_pool(name="xin", bufs=4))
        xbpool = es.enter_context(tc.tile_pool(name="xbf", bufs=3))
        epool = es.enter_context(tc.tile_pool(name="esb", bufs=3))
        smpool = es.enter_context(tc.tile_pool(name="small", bufs=8))
        apool = es.enter_context(tc.tile_pool(name="attb", bufs=4))
        vpool = es.enter_context(tc.tile_pool(name="vprime", bufs=6))
        ppool = es.enter_context(tc.tile_pool(name="prod", bufs=7))
        ypool = es.enter_context(tc.tile_pool(name="ysb", bufs=2))
        ytpool = es.enter_context(tc.tile_pool(name="ytsb", bufs=4))
        fpool = es.enter_context(tc.tile_pool(name="fsb", bufs=2))
        transps = es.enter_context(tc.tile_pool(name="tps", bufs=2, space="PSUM"))
        attps = es.enter_context(tc.tile_pool(name="attps", bufs=1, space="PSUM"))
        ups = es.enter_context(tc.tile_pool(name="ups", bufs=3, space="PSUM"))
        ypreps = es.enter_context(tc.tile_pool(name="ypreps", bufs=1, space="PSUM"))
        finps = es.enter_context(tc.tile_pool(name="finps", bufs=1, space="PSUM"))

        # ---- constants: DMA f32, cast to bf16 where needed.  Ordered so the
        # pipeline can start ASAP: shifts (eye for the first transpose) and
        # the first x rows go before the big weight tensors. ----
        def load_const_bf16(dram_ap, shape, name):
            st = spool.tile(shape, F32, tag=f"stage_{name}", name=f"stage_{name}")
            nc.sync.dma_start(out=st[:], in_=dram_ap)
            t = cpool.tile(shape, BF16, tag=name, name=name)
            nc.vector.tensor_copy(t[:], st[:])
            return t

        shifts = load_const_bf16(shifts_d[:], [W, 3 * W], "shifts")
        wqkvT = load_const_bf16(wqkvT_d[:], [C, O324], "wqkvT")
        wvT = load_const_bf16(wvT_d[:], [C, K2 * C], "wvT")
        wprojT = load_const_bf16(wprojT_d[:], [C, C], "wprojT")
        masks = cpool.tile([W, ROWS], F32, tag="masks")
        nc.sync.dma_start(out=masks[:], in_=masks_d[:])

        eye_bf = shifts[:, W : 2 * W]  # shift b=1 is the identity


        # persistent x-transpose tiles (manual rotation; edge columns are
        # zeroed once and never rewritten -> image border padding)
        xtp = [
            cpool.tile([C, W + 2], BF16, tag=f"xtp{i}", name=f"xtp{i}")
            for i in range(4)
        ]
        for i in range(4):
            nc.gpsimd.memset(xtp[i][:, 0:1], 0.0)
            nc.gpsimd.memset(xtp[i][:, W + 1 : W + 2], 0.0)

        scale = float(HD) ** -0.5

        # ---- flattened software pipeline across reps: rep r's virtual row
        # v = S - r*ROWS runs from -1 (prefetch) to ROWS+6 (tail folds), so
        # rep r's PE-only fold/proj tail overlaps rep r+1's DVE/Pool ramp ----
        x_tiles = {}
        v_tiles = {}
        prod_tiles = {}
        attb_tiles = {}
        ysb_tiles = {}
        ytsb_tiles = {}

        if True:  # keep indentation of the original rep-loop body
            def load_x(rep, s):
                x_sb = xpool.tile([W, C], F32, tag="x", name=f"x{rep}_{s}")
                nc.sync.dma_start(out=x_sb[:], in_=x_d[s])
                x_tiles[(rep, s)] = x_sb

            def cast_x(rep, s):
                xb = xbpool.tile([W, C], BF16, tag="xb", name=f"xb{rep}_{s}")
                nc.scalar.copy(xb[:], x_tiles.pop((rep, s))[:])
                return xb

            def transpose_x(s, xb):
                xt_ps = transps.tile([C, W], BF16, tag="tr")
                nc.tensor.transpose(xt_ps[:], xb[:], eye_bf)
                nc.scalar.copy(xtp[s % 4][:, 1 : W + 1], xt_ps[:])

            def get_vtile(rep, t):
                if (rep, t) not in v_tiles:
                    v_tiles[(rep, t)] = vpool.tile(
                        [W, HEADS * K2 * HD], BF16, tag="vp", name=f"vp{rep}_{t}"
                    )
                return v_tiles[(rep, t)]

            def u_matmuls(rep, s):
                # q = a*3 + b ; contributes to mult-row t = s - a + 1
                # One PSUM tile per a (3 q's) so each evacuates in one ACT op.
                xs = xtp[s % 4]
                u_ts = [
                    ups.tile([W, 3 * C], F32, tag="u", name=f"u{rep}_{s}_{a_}")
                    for a_ in range(K)
                ]
                for b in (1, 0, 2):
                    for a in range(K):
                        t = s - a + 1
                        if not (1 <= t <= ROWS - 2):
                            continue
                        q = a * K + b
                        nc.tensor.matmul(
                            u_ts[a][:, b * C : (b + 1) * C],
                            xs[:, b : b + W],
                            wvT[:, q * C : (q + 1) * C],
                            start=True,
                            stop=True,
                        )
                for a in range(K):
                    t = s - a + 1
                    if not (1 <= t <= ROWS - 2):
                        continue
                    vt = get_vtile(rep, t)
                    vdst = vt[:].rearrange(
                        "p (h q d) -> p h q d", h=HEADS, q=K2, d=HD
                    )[:, :, 3 * a : 3 * a + 3, :]
                    usrc = u_ts[a][:].rearrange(
                        "p (q h d) -> p h q d", q=K, h=HEADS, d=HD
                    )
                    nc.scalar.copy(vdst, usrc)

            def att_row(rep, s):
                # PE scores -> ACT exp, written bf16 and duplicated (u=2) so
                # every downstream DVE op sees packed [1,2] innermost dims and
                # runs in 2x mode. (softmax tail is emitted in att_tail AFTER
                # the row's products so DVE/Pool queue heads never idle-wait
                # on same-row exp)
                xs = xtp[s % 4]
                att_ps = attps.tile([W, O324], F32, tag="att")
                nc.tensor.matmul(
                    att_ps[:], xs[:, 1 : W + 1], wqkvT[:], start=True, stop=True
                )
                e2 = epool.tile([W, O324 * 2], BF16, tag="e", name=f"e{rep}_{s}")
                nc.scalar.activation(
                    e2[:].rearrange("p (o u) -> p o u", u=2),
                    att_ps[:][:, :, None].broadcast_to([W, O324, 2]),
                    AF.Exp,
                    scale=scale,
                )
                return e2

            def att_tail(rep, s, e2):
                # DVE: sum over q (free-axis reduce on the u=0 lane),
                # reciprocal (bf16, duplicated), attb in 2x mode.
                ev = e2[:].rearrange("p (g q u) -> p g q u", q=K2, u=2)
                evu = e2[:].rearrange("p (g q u) -> p u g q", q=K2, u=2)
                ssum = smpool.tile([W, 36], F32, tag="ssum")
                nc.vector.tensor_reduce(
                    ssum[:],
                    evu[:, 0:1],
                    axis=mybir.AxisListType.X,
                    op=mybir.AluOpType.add,
                )
                recip2 = smpool.tile([W, 36 * 2], BF16, tag="recip")
                with nc.allow_low_precision(reason="softmax recip bf16; tol 2e-2"):
                    nc.vector.reciprocal(
                        recip2[:].rearrange("p (g u) -> p g u", u=2),
                        ssum[:][:, :, None].broadcast_to([W, 36, 2]),
                    )
                if s in (1, ROWS - 2):
                    # image top/bottom: zero att rows outside the image
                    # (only these rows can be out of range on any core)
                    recipm2 = smpool.tile([W, 36 * 2], BF16, tag="recipm")
                    nc.vector.tensor_scalar_mul(
                        recipm2[:], recip2[:], masks[:, s : s + 1]
                    )
                    recip2 = recipm2
                attb = apool.tile([W, 36 * K2 * 2], BF16, tag="attb",
                                  name=f"attb{rep}_{s}")
                nc.vector.tensor_tensor(
                    attb[:].rearrange("p (g q u) -> p g q u", g=36, q=K2, u=2),
                    ev,
                    recip2[:]
                    .rearrange("p (g u) -> p g u", u=2)[:, :, None, :]
                    .broadcast_to([W, 36, K2, 2]),
                    op=mybir.AluOpType.mult,
                )
                attb_tiles[(rep, s)] = attb

            def products(rep, m):
                # prod[px, h, k, q, e, u] = att * V'.  DVE (bf16 2x mode,
                # 0.52 ns/elem) vs Pool (gpsimd TT, ~1.98 ns/elem): split so
                # both engines finish a row in ~5.2us.
                pt = ppool.tile([W, HEADS * K2 * K2 * HD], BF16, tag="prod",
                                name=f"prod{rep}_{m}")
                prod_tiles[(rep, m)] = pt
                vv = v_tiles[(rep, m)][:].rearrange(
                    "p (h q e u) -> p h q e u", h=HEADS, q=K2, e=HD // 2, u=2
                )
                av = attb_tiles[(rep, m)][:].rearrange(
                    "p (h k q u) -> p h k q u", h=HEADS, k=K2, q=K2, u=2
                )
                pv = pt[:].rearrange(
                    "p (h k q e u) -> p h k q e u",
                    h=HEADS, k=K2, q=K2, e=HD // 2, u=2,
                )

                def emit(eng, h, k0, k1):
                    kn = k1 - k0
                    a_b = av[:, h, k0:k1][:, :, :, None, :].broadcast_to(
                        [W, kn, K2, HD // 2, 2]
                    )
                    v_b = vv[:, h][:, None, :, :, :].broadcast_to(
                        [W, kn, K2, HD // 2, 2]
                    )
                    eng.tensor_tensor(
                        pv[:, h, k0:k1], a_b, v_b, op=mybir.AluOpType.mult
                    )

                # Unit = one (h, k) pair (288 elems). Pool takes the last
                # POOL_UNITS units; DVE the rest.
                dve_until = HEADS * K2 - POOL_UNITS
                for h in range(HEADS):
                    lo, hi = h * K2, h * K2 + K2
                    d_hi = min(hi, dve_until)
                    if d_hi > lo:
                        emit(nc.vector, h, 0, d_hi - lo)
                    p_lo = max(lo, dve_until)
                    if hi > p_lo:
                        emit(nc.gpsimd, h, p_lo - lo, K2)

            def fold_group(rep, i0, nrows):
                # fold + q-reduce: 27*(nrows+2) shift matmuls into one PSUM
                # region; first matmul is full-width (start=True covers all
                # row blocks)
                ypre_ps = ypreps.tile([W, nrows * C], F32, tag="ypre",
                                      name=f"ypre{rep}_{i0}")
                ts_ = sorted(
                    range(i0 - 1, i0 + nrows + 1),
                    key=lambda t_: -min(i0 + nrows - 1, t_ + 1) + max(i0, t_ - 1),
                )
                tinfos = []
                for t in ts_:
                    jlo = max(i0, t - 1)
                    jhi = min(i0 + nrows - 1, t + 1)
                    if jlo > jhi or not (1 <= t <= ROWS - 2):
                        continue
                    pv6 = prod_tiles[(rep, t)][:].rearrange(
                        "p (h a b q d) -> p a h b q d",
                        h=HEADS, a=K, b=K, q=K2, d=HD,
                    )
                    tinfos.append((t, jlo, jlo - t + 1, jhi - jlo + 1, pv6))
                # t-major (so PE pipelines against products still finishing
                # on Pool for the newest t), with a palindromic b order so
                # consecutive t-blocks share their boundary shift stationary
                # (LDW dedup: 15 -> 11 loads per group; LDW is free in the
                # cost model but ~107ns each on real HW)
                mms = []
                for ti, (t, jlo, a0, alen, pv6) in enumerate(tinfos):
                    border = (0, 1, 2) if ti % 2 == 0 else (2, 1, 0)
                    for b1 in border:
                        for q in range(K2):
                            mms.append((t, jlo, a0, alen, b1, q, pv6))
                for n_, (t, jlo, a0, alen, b1, q, pv6) in enumerate(mms):
                    rhs = pv6[:, a0 : a0 + alen, :, b1, q, :]
                    nc.tensor.matmul(
                        ypre_ps[:, (jlo - i0) * C : (jlo - i0 + alen) * C],
                        shifts[:, b1 * W : (b1 + 1) * W],
                        rhs,
                        start=(n_ == 0),
                        stop=(n_ == len(mms) - 1),
                    )
                ypre_sb = ypool.tile([W, nrows * C], BF16, tag="ypre_sb",
                                     name=f"ypre_sb{rep}_{i0}")
                nc.scalar.copy(ypre_sb[:], ypre_ps[:])
                ysb_tiles[(rep, i0)] = (ypre_sb, nrows)

            def yt_transposes(rep, i0):
                ypre_sb, nrows = ysb_tiles[(rep, i0)]
                yts = []
                for r_ in range(nrows):
                    yt_ps = transps.tile([C, W], BF16, tag="tr")
                    nc.tensor.transpose(
                        yt_ps[:], ypre_sb[:, r_ * C : (r_ + 1) * C], eye_bf
                    )
                    yt_sb = ytpool.tile([C, W], BF16, tag="yt_sb")
                    nc.scalar.copy(yt_sb[:], yt_ps[:])
                    yts.append(yt_sb)
                ytsb_tiles[(rep, i0)] = yts

            def proj_group(rep, i0):
                yts = ytsb_tiles.pop((rep, i0))
                nrows = len(yts)
                fin_ps = finps.tile([W, nrows * C], F32, tag="fin",
                                    name=f"fin{rep}_{i0}")
                for r_, yt_sb in enumerate(yts):
                    nc.tensor.matmul(
                        fin_ps[:, r_ * C : (r_ + 1) * C],
                        yt_sb[:],
                        wprojT[:],
                        start=True,
                        stop=True,
                    )
                fin_sb = fpool.tile([W, nrows * C], F32, tag="fin_sb",
                                    name=f"fin_sb{rep}_{i0}")
                nc.scalar.copy(fin_sb[:], fin_ps[:])
                dst = out_d[i0 - 2 : i0 - 2 + nrows]
                nc.sync.dma_start(
                    out=dst.rearrange("r w c -> w r c"),
                    in_=fin_sb[:].rearrange("w (r c) -> w r c", r=nrows),
                )

            def emit_step(rep, v):
                # one virtual pipeline step of rep: v in [-1, ROWS+6]
                if v == -1:
                    # prologue: prefetch + first transpose
                    load_x(rep, 0)
                    load_x(rep, 1)
                    transpose_x(0, cast_x(rep, 0))
                    return
                if v + 2 < ROWS:
                    load_x(rep, v + 2)
                xb_next = cast_x(rep, v + 1) if v + 1 < ROWS else None
                # PE: att + U first (inputs one row old)
                e2_row = None
                if 1 <= v <= ROWS - 2:
                    e2_row = att_row(rep, v)
                if v < ROWS:
                    u_matmuls(rep, v)
                # PE: lagged y-transposes (before x-transpose: transps PSUM
                # slots rotate yt0,yt1,yt2,xT with prompt ACT evacuations)
                g = v - 6
                fold_due = v >= 8 and (v - 8) % 3 == 0
                if fold_due and 5 <= g and g - 3 <= ROWS - 3:
                    yt_transposes(rep, g - 3)
                if xb_next is not None:
                    transpose_x(v + 1, xb_next)
                # DVE/Pool: products for mult-row m = v - 2
                m = v - 2
                if 1 <= m <= ROWS - 2:
                    products(rep, m)
                # PE: this row's fold, then lagged projection
                if fold_due and g <= ROWS - 3:
                    fold_group(rep, g, 3 if g <= ROWS - 6 else 1)
                if fold_due and 5 <= g and g - 3 <= ROWS - 3:
                    proj_group(rep, g - 3)
                # softmax tail (after products in DVE/Pool queues)
                if e2_row is not None:
                    att_tail(rep, v, e2_row)

            # reps are spaced ROWS-1 steps apart: the next rep's ramp (3 rows
            # with no DVE/Pool product work) overlaps one extra tail step of
            # the previous rep, squeezing out the remaining seam bubble
            PHASE = ROWS - 1
            for S in range(-1, (repeat - 1) * PHASE + ROWS + 7):
                for rep in range(repeat):
                    v = S - rep * PHASE
                    if -1 <= v <= ROWS + 6:
                        emit_step(rep, v)

    _dedup_ldweights(nc)
    _split_multi_waits(nc)
    return nc


def _dedup_ldweights(nc):
    """Delete InstLdweights whose weights AP is identical to the previous
    weight load on the PE stream (weights persist in the array). Transposes
    load their own stationary, so they invalidate the tracked state. Waits on
    a deleted LDW move to the next kept instruction."""
    import concourse.mybir as mb

    def apkey(arg):
        t = getattr(arg, "bass_ap", None)
        if t is None:
            return str(arg)
        return (t.tensor.name, t.offset, tuple(map(tuple, t.ap)))

    for f in nc.m.functions:
        for bb in f.blocks:
            last_key = None
            pending_waits = []
            out = []
            for inst in bb.instructions:
                eng = str(getattr(inst, "engine", ""))
                tname = type(inst).__name__
                if not eng.endswith("PE"):
                    out.append(inst)
                    continue
                if tname == "InstLdweights":
                    key = tuple(apkey(a) for a in inst.ins)
                    if key == last_key:
                        si = inst.sync_info
                        if si is not None and si.on_wait:
                            pending_waits.extend(si.on_wait)
                        continue
                    last_key = key
                elif tname == "InstMatmult":
                    if getattr(inst, "is_transpose", False):
                        last_key = None
                else:
                    last_key = None
                if pending_waits:
                    si = inst.sync_info
                    if si is None:
                        inst.sync_info = mb.SyncInfo(
                            on_wait=list(pending_waits), on_update=[]
                        )
                    else:
                        si.on_wait = list(pending_waits) + list(si.on_wait)
                    pending_waits = []
                out.append(inst)
            assert not pending_waits
            bb.instructions[:] = out


def _split_multi_waits(nc, limit=1):
    """Walrus codegen accepts at most one sync-wait per instruction on some
    engine structs. Split extras into same-engine NoOps preceding the
    instruction (in-order queues make sequential waits equivalent)."""
    nid = [0]

    def mknop(inst, wait):
        nid[0] += 1
        return mybir.InstNoOp(
            name=f"I-waitnop-{nid[0]}",
            engine=inst.engine,
            ins=[],
            outs=[],
            sync_info=mybir.SyncInfo(on_wait=[wait], on_update=[]),
        )

    for f in nc.m.functions:
        for bb in f.blocks:
            out = []
            for inst in bb.instructions:
                si = inst.sync_info
                if si is not None and si.on_wait and len(si.on_wait) > limit:
                    waits = list(si.on_wait)
                    for w in waits[:-limit]:
                        out.append(mknop(inst, w))
                    si.on_wait = waits[-limit:]
                out.append(inst)
            bb.instructions[:] = out


def prep_inputs(x, w_qkv, w_v, w_proj):
    """Host-side input prep -> per-core input maps."""
    wqkvT = np.ascontiguousarray(w_qkv.T).astype(np.float32)  # [C, 324]
    # wvT[j, q*C + c] = w_v[q, c, j]
    wvT = np.ascontiguousarray(
        np.transpose(w_v, (2, 0, 1)).reshape(C, K2 * C)
    ).astype(np.float32)
    wprojT = np.ascontiguousarray(w_proj.T).astype(np.float32)  # [c, o]
    # S_b[n', j] = delta(n' == j - b + 1) = eye(k = b - 1)
    shifts = np.concatenate(
        [np.eye(W, k=b - 1, dtype=np.float32) for b in range(3)], axis=1
    )

    in_maps = []
    for core in range(N_CORES):
        bb = core // 2
        half = core % 2
        r0 = half * (H // 2)
        # rows r0-2 .. r0+65 with zero pad outside image
        xs = np.zeros((ROWS, W, C), np.float32)
        lo = max(0, r0 - 2)
        hi = min(H, r0 + H // 2 + 2)
        xs[lo - (r0 - 2) : hi - (r0 - 2)] = x[bb, lo:hi]
        # mask: shard row s = image row r0 - 2 + s ; valid iff 0 <= row < H
        mk = np.zeros((ROWS,), np.float32)
        rows = r0 - 2 + np.arange(ROWS)
        mk[(rows >= 0) & (rows < H)] = 1.0
        masks = np.ascontiguousarray(np.broadcast_to(mk[None, :], (W, ROWS)))
        in_maps.append(
            {
                "x": xs,
                "wqkvT": wqkvT,
                "wvT": wvT,
                "wprojT": wprojT,
                "shifts": shifts,
                "masks": masks,
            }
        )
    return in_maps


def kernel(x, w_qkv, w_v, w_proj, _trace=False):
    global LAST_RESULTS
    if "nc" not in _CACHE:
        _CACHE["nc"] = build_graph()
    nc = _CACHE["nc"]
    in_maps = prep_inputs(
        np.asarray(x, np.float32),
        np.asarray(w_qkv, np.float32),
        np.asarray(w_v, np.float32),
        np.asarray(w_proj, np.float32),
    )
    res = run_bass_kernel_spmd(nc, in_maps, list(range(N_CORES)), trace=_trace)
    LAST_RESULTS = res
    y = np.zeros((B, H, W, C), np.float32)
    for core in range(N_CORES):
        bb = core // 2
        half = core % 2
        r0 = half * (H // 2)
        y[bb, r0 : r0 + H // 2] = res.results[core]["out"]
    return y

